# revision 20
# baseline (speedup 1.0000x reference)
"""ConvCNP encoder kernel for 8 Trainium2 NeuronCores.

Computes, for full inputs X(4,1024,2), Y(4,1024,2), grid(16384,2):
    Gram = exp(-0.5*||grid-X||^2)          (B, G, n)
    FM   = Gram @ [1, Y]                   (B, G, 3)
    out  = [FM0, FM1/FM0, FM2/FM0] -> (B, 3, 128, 128)  (y, x image axes)

Sharding: grid axis G split 8 ways (2048 rows / core = 16 output
x-columns); every core handles all 4 batches; no cross-device reduction.

Per-core device pipeline:
  mm1 (PE):  -0.5*d2 as K=10 bf16 matmuls using a hi/lo split of
             g.x - 0.5|g|^2 - 0.5|x|^2 -> PSUM [n-tile 128, g 512].
             K=10 << 128, so 4 matmuls run concurrently in 32-row
             strips of the PE array via tile_position (inputs are
             replicated at partition offsets 0/32/64/96).
  exp (ACT): PSUM -> SBUF Gram (bf16), batched [128, <=4*512] exps
  mm2 (PE):  Gram[n,g128] stationary x E[n, (1,Yhi,Ylo)] moving,
             accumulated over 8 n-tiles -> FM [g(=y) 128, 5] PSUM
  norm (DVE): density reciprocal + multiplies, fp32
  DMA out:   [y, x] tiles per (b, c)
"""

import numpy as np

B = 4
N = 1024
G = 16384
NCORES = 8
GS = G // NCORES          # 2048 grid rows per core
NT = N // 128             # 8 context tiles
JS = GS // 512            # 4 g-blocks of 512 per core
K = 10                    # contraction rows of the d2 factorization
XCOLS = GS // 128         # 16 output x-columns per core

# combined input layout (free-dim offsets, bf16 elements)
A_W = B * 2 * 128         # 1024
B_W = GS                  # 2048
E_W = B * NT * 5          # 160
IN_W = A_W + B_W + E_W    # 3232

_CACHE = {}


def _build_nc():
    import concourse.bacc as bacc
    import concourse.mybir as mybir
    import concourse.tile as tile
    from contextlib import ExitStack

    f32 = mybir.dt.float32
    bf16 = mybir.dt.bfloat16

    nc = bacc.Bacc("TRN2", target_bir_lowering=False, debug=False,
                   num_devices=NCORES)
    in_d = nc.dram_tensor("IN", [128, IN_W], bf16, kind="ExternalInput")
    out_d = nc.dram_tensor("OUT", [B, 3, 128, XCOLS], f32, kind="ExternalOutput")

    EXP = mybir.ActivationFunctionType.Exp

    with tile.TileContext(nc) as tc, ExitStack() as ctx:
        consts = ctx.enter_context(tc.tile_pool(name="consts", bufs=1))
        gram_pool = ctx.enter_context(tc.tile_pool(name="gram", bufs=4))
        mm1a_pool = ctx.enter_context(tc.tile_pool(name="mm1a", bufs=1, space="PSUM"))
        mm1b_pool = ctx.enter_context(tc.tile_pool(name="mm1b", bufs=1, space="PSUM"))
        mm2_pool = ctx.enter_context(tc.tile_pool(name="mm2", bufs=1, space="PSUM"))
        small = ctx.enter_context(tc.tile_pool(name="small", bufs=4))
        outp = ctx.enter_context(tc.tile_pool(name="outp", bufs=1))

        # separate tiles so consumers only wait for the DMA they need;
        # the first mm1 group touches only A[b0] and B[j0] — land those
        # first, one on each queue, before the bulk.
        a0_sb = consts.tile([128, 2 * 128], bf16)
        a123_sb = consts.tile([128, 3 * 2 * 128], bf16)
        b_t = [consts.tile([128, 512], bf16, name=f"bj{j}", tag=f"bj{j}")
               for j in range(JS)]
        e_sb = consts.tile([128, E_W], bf16)

        def in_col(c0, w):
            return in_d[:, c0:c0 + w]

        nc.sync.dma_start(out=b_t[0], in_=in_col(A_W, 512))
        nc.gpsimd.dma_start(out=a0_sb, in_=in_col(0, 256))
        nc.gpsimd.dma_start(out=b_t[1], in_=in_col(A_W + 512, 512))
        nc.sync.dma_start(out=a123_sb, in_=in_col(256, 768))
        nc.sync.dma_start(out=b_t[2], in_=in_col(A_W + 1024, 512))
        nc.gpsimd.dma_start(out=b_t[3], in_=in_col(A_W + 1536, 512))
        nc.sync.dma_start(out=e_sb, in_=in_col(A_W + B_W, E_W))

        # views: A[b] -> [128, 2, 128], E [128, B, NT, 5]
        a0_v = a0_sb.rearrange("p (h m) -> p h m", h=2)
        a123_v = a123_sb.rearrange("p (b h m) -> p b h m", b=3, h=2)
        e_v = e_sb.rearrange("p (b t c) -> p b t c", b=B, t=NT)

        def a_slice(b, row, h4):
            if b == 0:
                return a0_v[32 * row:32 * row + K, h4, :]
            return a123_v[32 * row:32 * row + K, b - 1, h4, :]

        def b_slice(j):
            return b_t[j]

        out_sb = outp.tile([128, B, 3, XCOLS], f32)
        grams = {}

        def emit_mm1_group(b, slots, ps):
            """slots: global slot ids (b-local, 0..31), j = s//8, nt = s%8"""
            for i, s in enumerate(slots):
                j = s // 8
                nt = s % 8
                row = nt % 4
                lhsT = a_slice(b, row, nt // 4)
                rhs = b_slice(j)[32 * row:32 * row + K, :]
                nc.tensor.matmul(ps[:, i, :], lhsT, rhs,
                                 start=True, stop=True,
                                 tile_position=(32 * row, 0))

        def emit_mm1_exp(b, h, sizes, tags):
            """one gram tile covering the half's 16 slots (j-outer)"""
            gram = gram_pool.tile([128, 16, 512], bf16, tag="gram",
                                  name=f"gram{b}{h}")
            grams[(b, h)] = gram
            s0 = 0
            for gsz, sel in zip(sizes, tags):
                pool = (mm1a_pool, mm1b_pool)[sel]
                cap = (4, 3)[sel]
                ps = pool.tile([128, cap, 512], f32, tag=f"t{sel}",
                               name=f"ps{sel}")
                emit_mm1_group(b, [16 * h + s0 + i for i in range(gsz)], ps)
                nc.scalar.activation(out=gram[:, s0:s0 + gsz, :],
                                     in_=ps[:, 0:gsz, :], func=EXP)
                s0 += gsz

        def emit_mm1_exp_perj(b, j, sizes, tags):
            """per-j gram tile (8 slots) — used for the final half so the
            tail mm2 only waits on its own j block"""
            gram = gram_pool.tile([128, 8, 512], bf16, tag="gramj",
                                  name=f"gramj{b}{j}")
            grams[("j", b, j)] = gram
            s0 = 0
            for gsz, sel in zip(sizes, tags):
                pool = (mm1a_pool, mm1b_pool)[sel]
                cap = (4, 3)[sel]
                ps = pool.tile([128, cap, 512], f32, tag=f"t{sel}",
                               name=f"ps{sel}")
                emit_mm1_group(b, [8 * j + s0 + i for i in range(gsz)], ps)
                nc.scalar.activation(out=gram[:, s0:s0 + gsz, :],
                                     in_=ps[:, 0:gsz, :], func=EXP)
                s0 += gsz

        def emit_mm2_j(b, j, gram, base):
            fm = grams[("fm", b)]
            for r in range(4):
                gsub = j * 4 + r
                for nt in range(NT):
                    nc.tensor.matmul(
                        fm[:, gsub, :],
                        gram[:, base + nt, r * 128:(r + 1) * 128],
                        e_v[:, b, nt, :],
                        start=(nt == 0),
                        stop=(nt == NT - 1),
                    )

        def emit_norm(b, sl, dma_engine):
            """normalize fm[:, sl, :] and emit ONE combined output DMA
            covering all 3 channels of this gsub range"""
            fm = grams[("fm", b)]
            w = sl.stop - sl.start
            fmc = small.tile([128, 8, 5], f32, tag="fmc")
            nc.vector.tensor_copy(fmc[:, 0:w, :], fm[:, sl, :])
            recip = small.tile([128, 8], f32, tag="recip")
            nc.vector.reciprocal(recip[:, 0:w], fmc[:, 0:w, 0])
            nc.vector.tensor_copy(out_sb[:, b, 0, sl], fmc[:, 0:w, 0])
            v1 = small.tile([128, 8], f32, tag="v1")
            nc.vector.tensor_add(v1[:, 0:w], fmc[:, 0:w, 1], fmc[:, 0:w, 3])
            nc.vector.tensor_mul(out_sb[:, b, 1, sl], v1[:, 0:w], recip[:, 0:w])
            v2 = small.tile([128, 8], f32, tag="v2")
            nc.vector.tensor_add(v2[:, 0:w], fmc[:, 0:w, 2], fmc[:, 0:w, 4])
            nc.vector.tensor_mul(out_sb[:, b, 2, sl], v2[:, 0:w], recip[:, 0:w])
            # dst iterated (y, c, x) to match the SBUF (partition, c, x) order
            dst = out_d[b, :, :, sl].rearrange("c y x -> y c x")
            dma_engine.dma_start(out=dst, in_=out_sb[:, b, :, sl])

        # pattern schedule: first half starts with a 2-slot group so the
        # first exp fires as early as possible; thereafter tags alternate
        # across the boundary so ACT never waits for a psum refill.
        half_patterns = [((2, 4, 3, 4, 3), (1, 0, 1, 0, 1))]
        for k in range(1, 7):
            if k % 2 == 1:
                half_patterns.append(((4, 3, 4, 3, 2), (0, 1, 0, 1, 0)))
            else:
                half_patterns.append(((3, 4, 3, 4, 2), (1, 0, 1, 0, 1)))

        # software pipeline: mm1/exp of (b) overlaps mm2/norm of (b-1)
        for b in range(B):
            fm_t = mm2_pool.tile([128, XCOLS, 5], f32, tag="fm")
            grams[("fm", b)] = fm_t
            if b < B - 1:
                emit_mm1_exp(b, 0, *half_patterns[2 * b])
                emit_mm1_exp(b, 1, *half_patterns[2 * b + 1])
            else:
                emit_mm1_exp(b, 0, *half_patterns[6])
                # final half: per-j grams; tag sequence continues from the
                # half-6 pattern which ends on tag b -> start tag a
                emit_mm1_exp_perj(b, 2, (4, 3, 1), (0, 1, 0))
                emit_mm1_exp_perj(b, 3, (3, 4, 1), (1, 0, 1))
            if b >= 1:
                p = b - 1
                for h in range(2):
                    g = grams[(p, h)]
                    emit_mm2_j(p, 2 * h, g, 0)
                    emit_mm2_j(p, 2 * h + 1, g, 8)
                    emit_norm(p, slice(8 * h, 8 * h + 8),
                              (nc.sync, nc.gpsimd)[h])
        b = B - 1
        g = grams[(b, 0)]
        emit_mm2_j(b, 0, g, 0)
        emit_mm2_j(b, 1, g, 8)
        emit_norm(b, slice(0, 8), nc.sync)
        emit_mm2_j(b, 2, grams[("j", b, 2)], 0)
        emit_norm(b, slice(8, 12), nc.gpsimd)
        emit_mm2_j(b, 3, grams[("j", b, 3)], 0)
        emit_norm(b, slice(12, 16), nc.sync)

    nc.compile()
    return nc


def _split_hi_lo(a):
    import ml_dtypes

    bf = ml_dtypes.bfloat16
    hi = a.astype(bf).astype(np.float32)
    lo = (a - hi).astype(bf).astype(np.float32)
    return hi, lo


def _prepare_inputs(X, Y, grid):
    """Host-side packing: per-core input maps for the SPMD kernel."""
    import ml_dtypes

    bf = ml_dtypes.bfloat16
    X = np.asarray(X, np.float32)
    Y = np.asarray(Y, np.float32)
    grid = np.asarray(grid, np.float32)

    sx = -0.5 * np.sum(X * X, axis=-1)        # (B, N)
    sg = -0.5 * np.sum(grid * grid, axis=-1)  # (G,)
    xh, xl = _split_hi_lo(X)
    gh, gl = _split_hi_lo(grid)
    sxh, sxl = _split_hi_lo(sx)
    sgh, sgl = _split_hi_lo(sg)
    ones_n = np.ones((B, N), np.float32)
    ones_g = np.ones((G,), np.float32)

    # M'[n, g] = sum_k A[k, n] * Bm[k, g] = g.x - 0.5|x|^2 - 0.5|g|^2
    A = np.stack(
        [xh[..., 0], xh[..., 1], xl[..., 0], xl[..., 1],
         xh[..., 0], xh[..., 1], sxh, sxl, ones_n, ones_n],
        axis=1,
    )  # (B, K, N)
    Bm = np.stack(
        [gh[:, 0], gh[:, 1], gh[:, 0], gh[:, 1],
         gl[:, 0], gl[:, 1], ones_g, ones_g, sgh, sgl],
        axis=0,
    )  # (K, G)

    # A replicated into 4 row-strips: strip i (partitions 32i..32i+9)
    # holds A rows for nt = h*4 + i  -> [128, B, 2, 128]
    A4 = A.transpose(1, 0, 2).reshape(K, B, 2, 4, 128)  # k b h i p
    arep = np.zeros((128, B, 2, 128), np.float32)
    for i in range(4):
        arep[32 * i:32 * i + K] = A4[:, :, :, i, :]

    # E: [128, B, NT, 5] = [1, yh0, yh1, yl0, yl1]
    yh, yl = _split_hi_lo(Y)
    E = np.stack([ones_n, yh[..., 0], yh[..., 1], yl[..., 0], yl[..., 1]],
                 axis=-1)
    ey = E.reshape(B, NT, 128, 5).transpose(2, 0, 1, 3)

    in_maps = []
    for c in range(NCORES):
        # B replicated into the same 4 row-strips
        brep = np.zeros((128, GS), np.float32)
        for i in range(4):
            brep[32 * i:32 * i + K] = Bm[:, c * GS:(c + 1) * GS]
        packed = np.concatenate(
            [arep.reshape(128, A_W), brep, ey.reshape(128, E_W)], axis=1)
        in_maps.append({"IN": np.ascontiguousarray(packed).astype(bf)})
    return in_maps


def _run(in_maps, trace=False):
    from concourse.bass_utils import run_bass_kernel_spmd

    if "nc" not in _CACHE:
        _CACHE["nc"] = _build_nc()
    nc = _CACHE["nc"]
    return run_bass_kernel_spmd(nc, in_maps, core_ids=list(range(NCORES)),
                                trace=trace)


def kernel(X, Y, grid, _trace=False, _results_out=None):
    in_maps = _prepare_inputs(X, Y, grid)
    res = _run(in_maps, trace=_trace)
    out = np.empty((B, 3, 128, 128), np.float32)
    for c in range(NCORES):
        out[:, :, :, c * XCOLS:(c + 1) * XCOLS] = res.results[c]["OUT"]
    if _results_out is not None:
        _results_out.append(res)
    return out


# revision 21
# speedup vs baseline: 1.0045x; 1.0045x over previous
"""ConvCNP encoder kernel for 8 Trainium2 NeuronCores.

Computes, for full inputs X(4,1024,2), Y(4,1024,2), grid(16384,2):
    Gram = exp(-0.5*||grid-X||^2)          (B, G, n)
    FM   = Gram @ [1, Y]                   (B, G, 3)
    out  = [FM0, FM1/FM0, FM2/FM0] -> (B, 3, 128, 128)  (y, x image axes)

Sharding: grid axis G split 8 ways (2048 rows / core = 16 output
x-columns); every core handles all 4 batches; no cross-device reduction.

Per-core device pipeline:
  mm1 (PE):  -0.5*d2 as K=10 bf16 matmuls using a hi/lo split of
             g.x - 0.5|g|^2 - 0.5|x|^2 -> PSUM [n-tile 128, g 512].
             K=10 << 128, so 4 matmuls run concurrently in 32-row
             strips of the PE array via tile_position (inputs are
             replicated at partition offsets 0/32/64/96).
  exp (ACT): PSUM -> SBUF Gram (bf16), batched [128, <=4*512] exps
  mm2 (PE):  Gram[n,g128] stationary x E[n, (1,Yhi,Ylo)] moving,
             accumulated over 8 n-tiles -> FM [g(=y) 128, 5] PSUM
  norm (DVE): density reciprocal + multiplies, fp32
  DMA out:   [y, x] tiles per (b, c)
"""

import numpy as np

B = 4
N = 1024
G = 16384
NCORES = 8
GS = G // NCORES          # 2048 grid rows per core
NT = N // 128             # 8 context tiles
JS = GS // 512            # 4 g-blocks of 512 per core
K = 10                    # contraction rows of the d2 factorization
XCOLS = GS // 128         # 16 output x-columns per core

# combined input layout (free-dim offsets, bf16 elements)
A_W = B * 2 * 128         # 1024
B_W = GS                  # 2048
E_W = B * NT * 5          # 160
IN_W = A_W + B_W + E_W    # 3232

_CACHE = {}


def _build_nc():
    import concourse.bacc as bacc
    import concourse.mybir as mybir
    import concourse.tile as tile
    from contextlib import ExitStack

    f32 = mybir.dt.float32
    bf16 = mybir.dt.bfloat16

    nc = bacc.Bacc("TRN2", target_bir_lowering=False, debug=False,
                   num_devices=NCORES)
    in_d = nc.dram_tensor("IN", [128, IN_W], bf16, kind="ExternalInput")
    out_d = nc.dram_tensor("OUT", [B, 3, 128, XCOLS], f32, kind="ExternalOutput")

    EXP = mybir.ActivationFunctionType.Exp

    with tile.TileContext(nc) as tc, ExitStack() as ctx:
        consts = ctx.enter_context(tc.tile_pool(name="consts", bufs=1))
        gram_pool = ctx.enter_context(tc.tile_pool(name="gram", bufs=4))
        mm1a_pool = ctx.enter_context(tc.tile_pool(name="mm1a", bufs=1, space="PSUM"))
        mm1b_pool = ctx.enter_context(tc.tile_pool(name="mm1b", bufs=1, space="PSUM"))
        mm2_pool = ctx.enter_context(tc.tile_pool(name="mm2", bufs=1, space="PSUM"))
        small = ctx.enter_context(tc.tile_pool(name="small", bufs=4))
        outp = ctx.enter_context(tc.tile_pool(name="outp", bufs=1))

        # separate tiles so consumers only wait for the DMA they need;
        # the first mm1 group touches only A[b0] and B[j0] — land those
        # first, one on each queue, before the bulk.
        a0_sb = consts.tile([128, 2 * 128], bf16)
        a123_sb = consts.tile([128, 3 * 2 * 128], bf16)
        b_t = [consts.tile([128, 512], bf16, name=f"bj{j}", tag=f"bj{j}")
               for j in range(JS)]
        e_sb = consts.tile([128, E_W], bf16)

        def in_col(c0, w):
            return in_d[:, c0:c0 + w]

        # IN column layout: [a0 256 | bj0 512 | bj1 512 | a123 768 |
        #                    bj2 512 | bj3 512 | e 160]
        nc.sync.dma_start(out=a0_sb, in_=in_col(0, 256))
        nc.sync.dma_start(out=b_t[0], in_=in_col(256, 512))
        nc.gpsimd.dma_start(out=b_t[1], in_=in_col(768, 512))
        nc.sync.dma_start(out=a123_sb, in_=in_col(1280, 768))
        nc.gpsimd.dma_start(out=b_t[2], in_=in_col(2048, 512))
        nc.sync.dma_start(out=b_t[3], in_=in_col(2560, 512))
        nc.gpsimd.dma_start(out=e_sb, in_=in_col(3072, E_W))

        # views: A[b] -> [128, 2, 128], E [128, B, NT, 5]
        a0_v = a0_sb.rearrange("p (h m) -> p h m", h=2)
        a123_v = a123_sb.rearrange("p (b h m) -> p b h m", b=3, h=2)
        e_v = e_sb.rearrange("p (b t c) -> p b t c", b=B, t=NT)

        def a_slice(b, row, h4):
            if b == 0:
                return a0_v[32 * row:32 * row + K, h4, :]
            return a123_v[32 * row:32 * row + K, b - 1, h4, :]

        def b_slice(j):
            return b_t[j]

        out_sb = outp.tile([128, B, 3, XCOLS], f32)
        grams = {}

        def emit_mm1_group(b, slots, ps):
            """slots: global slot ids (b-local, 0..31), j = s//8, nt = s%8"""
            for i, s in enumerate(slots):
                j = s // 8
                nt = s % 8
                row = nt % 4
                lhsT = a_slice(b, row, nt // 4)
                rhs = b_slice(j)[32 * row:32 * row + K, :]
                nc.tensor.matmul(ps[:, i, :], lhsT, rhs,
                                 start=True, stop=True,
                                 tile_position=(32 * row, 0))

        def emit_mm1_exp(b, h, sizes, tags):
            """one gram tile covering the half's 16 slots (j-outer)"""
            gram = gram_pool.tile([128, 16, 512], bf16, tag="gram",
                                  name=f"gram{b}{h}")
            grams[(b, h)] = gram
            s0 = 0
            for gsz, sel in zip(sizes, tags):
                pool = (mm1a_pool, mm1b_pool)[sel]
                cap = (4, 3)[sel]
                ps = pool.tile([128, cap, 512], f32, tag=f"t{sel}",
                               name=f"ps{sel}")
                emit_mm1_group(b, [16 * h + s0 + i for i in range(gsz)], ps)
                nc.scalar.activation(out=gram[:, s0:s0 + gsz, :],
                                     in_=ps[:, 0:gsz, :], func=EXP)
                s0 += gsz

        def emit_mm1_exp_perj(b, j, sizes, tags):
            """per-j gram tile (8 slots) — used for the final half so the
            tail mm2 only waits on its own j block"""
            gram = gram_pool.tile([128, 8, 512], bf16, tag="gramj",
                                  name=f"gramj{b}{j}")
            grams[("j", b, j)] = gram
            s0 = 0
            for gsz, sel in zip(sizes, tags):
                pool = (mm1a_pool, mm1b_pool)[sel]
                cap = (4, 3)[sel]
                ps = pool.tile([128, cap, 512], f32, tag=f"t{sel}",
                               name=f"ps{sel}")
                emit_mm1_group(b, [8 * j + s0 + i for i in range(gsz)], ps)
                nc.scalar.activation(out=gram[:, s0:s0 + gsz, :],
                                     in_=ps[:, 0:gsz, :], func=EXP)
                s0 += gsz

        def emit_mm2_j(b, j, gram, base):
            fm = grams[("fm", b)]
            for r in range(4):
                gsub = j * 4 + r
                for nt in range(NT):
                    nc.tensor.matmul(
                        fm[:, gsub, :],
                        gram[:, base + nt, r * 128:(r + 1) * 128],
                        e_v[:, b, nt, :],
                        start=(nt == 0),
                        stop=(nt == NT - 1),
                    )

        def emit_norm(b, sl, dma_engine):
            """normalize fm[:, sl, :] and emit ONE combined output DMA
            covering all 3 channels of this gsub range"""
            fm = grams[("fm", b)]
            w = sl.stop - sl.start
            fmc = small.tile([128, 8, 5], f32, tag="fmc")
            nc.vector.tensor_copy(fmc[:, 0:w, :], fm[:, sl, :])
            recip = small.tile([128, 8], f32, tag="recip")
            nc.vector.reciprocal(recip[:, 0:w], fmc[:, 0:w, 0])
            nc.vector.tensor_copy(out_sb[:, b, 0, sl], fmc[:, 0:w, 0])
            v1 = small.tile([128, 8], f32, tag="v1")
            nc.vector.tensor_add(v1[:, 0:w], fmc[:, 0:w, 1], fmc[:, 0:w, 3])
            nc.vector.tensor_mul(out_sb[:, b, 1, sl], v1[:, 0:w], recip[:, 0:w])
            v2 = small.tile([128, 8], f32, tag="v2")
            nc.vector.tensor_add(v2[:, 0:w], fmc[:, 0:w, 2], fmc[:, 0:w, 4])
            nc.vector.tensor_mul(out_sb[:, b, 2, sl], v2[:, 0:w], recip[:, 0:w])
            # dst iterated (y, c, x) to match the SBUF (partition, c, x) order
            dst = out_d[b, :, :, sl].rearrange("c y x -> y c x")
            dma_engine.dma_start(out=dst, in_=out_sb[:, b, :, sl])

        # pattern schedule: first half starts with a 2-slot group so the
        # first exp fires as early as possible; thereafter tags alternate
        # across the boundary so ACT never waits for a psum refill.
        half_patterns = [((2, 4, 3, 4, 3), (1, 0, 1, 0, 1))]
        for k in range(1, 7):
            if k % 2 == 1:
                half_patterns.append(((4, 3, 4, 3, 2), (0, 1, 0, 1, 0)))
            else:
                half_patterns.append(((3, 4, 3, 4, 2), (1, 0, 1, 0, 1)))

        # software pipeline: mm1/exp of (b) overlaps mm2/norm of (b-1)
        for b in range(B):
            fm_t = mm2_pool.tile([128, XCOLS, 5], f32, tag="fm")
            grams[("fm", b)] = fm_t
            if b < B - 1:
                emit_mm1_exp(b, 0, *half_patterns[2 * b])
                emit_mm1_exp(b, 1, *half_patterns[2 * b + 1])
            else:
                emit_mm1_exp(b, 0, *half_patterns[6])
                emit_mm1_exp(b, 1, ((4, 3, 4, 3, 2)), ((0, 1, 0, 1, 0)))
            if b >= 1:
                p = b - 1
                for h in range(2):
                    g = grams[(p, h)]
                    emit_mm2_j(p, 2 * h, g, 0)
                    emit_mm2_j(p, 2 * h + 1, g, 8)
                    emit_norm(p, slice(8 * h, 8 * h + 8), nc.sync)
        b = B - 1
        for h in range(2):
            g = grams[(b, h)]
            emit_mm2_j(b, 2 * h, g, 0)
            emit_mm2_j(b, 2 * h + 1, g, 8)
            emit_norm(b, slice(8 * h, 8 * h + 8), nc.sync)

    nc.compile()
    return nc


def _split_hi_lo(a):
    import ml_dtypes

    bf = ml_dtypes.bfloat16
    hi = a.astype(bf).astype(np.float32)
    lo = (a - hi).astype(bf).astype(np.float32)
    return hi, lo


def _prepare_inputs(X, Y, grid):
    """Host-side packing: per-core input maps for the SPMD kernel."""
    import ml_dtypes

    bf = ml_dtypes.bfloat16
    X = np.asarray(X, np.float32)
    Y = np.asarray(Y, np.float32)
    grid = np.asarray(grid, np.float32)

    sx = -0.5 * np.sum(X * X, axis=-1)        # (B, N)
    sg = -0.5 * np.sum(grid * grid, axis=-1)  # (G,)
    xh, xl = _split_hi_lo(X)
    gh, gl = _split_hi_lo(grid)
    sxh, sxl = _split_hi_lo(sx)
    sgh, sgl = _split_hi_lo(sg)
    ones_n = np.ones((B, N), np.float32)
    ones_g = np.ones((G,), np.float32)

    # M'[n, g] = sum_k A[k, n] * Bm[k, g] = g.x - 0.5|x|^2 - 0.5|g|^2
    A = np.stack(
        [xh[..., 0], xh[..., 1], xl[..., 0], xl[..., 1],
         xh[..., 0], xh[..., 1], sxh, sxl, ones_n, ones_n],
        axis=1,
    )  # (B, K, N)
    Bm = np.stack(
        [gh[:, 0], gh[:, 1], gh[:, 0], gh[:, 1],
         gl[:, 0], gl[:, 1], ones_g, ones_g, sgh, sgl],
        axis=0,
    )  # (K, G)

    # A replicated into 4 row-strips: strip i (partitions 32i..32i+9)
    # holds A rows for nt = h*4 + i  -> [128, B, 2, 128]
    A4 = A.transpose(1, 0, 2).reshape(K, B, 2, 4, 128)  # k b h i p
    arep = np.zeros((128, B, 2, 128), np.float32)
    for i in range(4):
        arep[32 * i:32 * i + K] = A4[:, :, :, i, :]

    # E: [128, B, NT, 5] = [1, yh0, yh1, yl0, yl1]
    yh, yl = _split_hi_lo(Y)
    E = np.stack([ones_n, yh[..., 0], yh[..., 1], yl[..., 0], yl[..., 1]],
                 axis=-1)
    ey = E.reshape(B, NT, 128, 5).transpose(2, 0, 1, 3)

    in_maps = []
    ar = arep.reshape(128, A_W)
    for c in range(NCORES):
        # B replicated into the same 4 row-strips
        brep = np.zeros((128, GS), np.float32)
        for i in range(4):
            brep[32 * i:32 * i + K] = Bm[:, c * GS:(c + 1) * GS]
        # column order: [a0 | bj0 | bj1 | a123 | bj2 | bj3 | e]
        packed = np.concatenate(
            [ar[:, 0:256], brep[:, 0:512], brep[:, 512:1024],
             ar[:, 256:A_W], brep[:, 1024:1536], brep[:, 1536:2048],
             ey.reshape(128, E_W)], axis=1)
        in_maps.append({"IN": np.ascontiguousarray(packed).astype(bf)})
    return in_maps


def _run(in_maps, trace=False):
    from concourse.bass_utils import run_bass_kernel_spmd

    if "nc" not in _CACHE:
        _CACHE["nc"] = _build_nc()
    nc = _CACHE["nc"]
    return run_bass_kernel_spmd(nc, in_maps, core_ids=list(range(NCORES)),
                                trace=trace)


def kernel(X, Y, grid, _trace=False, _results_out=None):
    in_maps = _prepare_inputs(X, Y, grid)
    res = _run(in_maps, trace=_trace)
    out = np.empty((B, 3, 128, 128), np.float32)
    for c in range(NCORES):
        out[:, :, :, c * XCOLS:(c + 1) * XCOLS] = res.results[c]["OUT"]
    if _results_out is not None:
        _results_out.append(res)
    return out


# revision 22
# speedup vs baseline: 2.2962x; 2.2860x over previous
"""ConvCNP encoder kernel for 8 Trainium2 NeuronCores.

Computes, for full inputs X(4,1024,2), Y(4,1024,2), grid(16384,2):
    Gram = exp(-0.5*||grid-X||^2)          (B, G, n)
    FM   = Gram @ [1, Y]                   (B, G, 3)
    out  = [FM0, FM1/FM0, FM2/FM0] -> (B, 3, 128, 128)  (y, x image axes)

Sharding: grid axis G split 8 ways (2048 rows / core = 16 output
x-columns); every core handles all 4 batches; no cross-device reduction.

Per-core device pipeline:
  mm1 (PE):  -0.5*d2 as K=10 bf16 matmuls using a hi/lo split of
             g.x - 0.5|g|^2 - 0.5|x|^2 -> PSUM [n-tile 128, g 512].
             K=10 << 128, so 4 matmuls run concurrently in 32-row
             strips of the PE array via tile_position (inputs are
             replicated at partition offsets 0/32/64/96).
  exp (ACT): PSUM -> SBUF Gram (bf16), batched [128, <=4*512] exps
  mm2 (PE):  Gram[n,g128] stationary x E[n, (1,Yhi,Ylo)] moving,
             accumulated over 8 n-tiles -> FM [g(=y) 128, 5] PSUM
  norm (DVE): density reciprocal + multiplies, fp32
  DMA out:   [y, x] tiles per (b, c)
"""

import numpy as np

B = 4
N = 1024
G = 16384
NCORES = 8
GS = G // NCORES          # 2048 grid rows per core
NT = N // 128             # 8 context tiles
JS = GS // 512            # 4 g-blocks of 512 per core
K = 10                    # contraction rows of the d2 factorization
XCOLS = GS // 128         # 16 output x-columns per core

# combined input layout (free-dim offsets, bf16 elements)
A_W = B * 2 * 128         # 1024
B_W = GS                  # 2048
E_W = B * NT * 5          # 160
IN_W = A_W + B_W + E_W    # 3232

_CACHE = {}


def _build_nc_general():
    import concourse.bacc as bacc
    import concourse.mybir as mybir
    import concourse.tile as tile
    from contextlib import ExitStack

    f32 = mybir.dt.float32
    bf16 = mybir.dt.bfloat16

    nc = bacc.Bacc("TRN2", target_bir_lowering=False, debug=False,
                   num_devices=NCORES)
    in_d = nc.dram_tensor("IN", [128, IN_W], bf16, kind="ExternalInput")
    out_d = nc.dram_tensor("OUT", [B, 3, 128, XCOLS], f32, kind="ExternalOutput")

    EXP = mybir.ActivationFunctionType.Exp

    with tile.TileContext(nc) as tc, ExitStack() as ctx:
        consts = ctx.enter_context(tc.tile_pool(name="consts", bufs=1))
        gram_pool = ctx.enter_context(tc.tile_pool(name="gram", bufs=4))
        mm1a_pool = ctx.enter_context(tc.tile_pool(name="mm1a", bufs=1, space="PSUM"))
        mm1b_pool = ctx.enter_context(tc.tile_pool(name="mm1b", bufs=1, space="PSUM"))
        mm2_pool = ctx.enter_context(tc.tile_pool(name="mm2", bufs=1, space="PSUM"))
        small = ctx.enter_context(tc.tile_pool(name="small", bufs=4))
        outp = ctx.enter_context(tc.tile_pool(name="outp", bufs=1))

        # separate tiles so consumers only wait for the DMA they need;
        # the first mm1 group touches only A[b0] and B[j0] — land those
        # first, one on each queue, before the bulk.
        a0_sb = consts.tile([128, 2 * 128], bf16)
        a123_sb = consts.tile([128, 3 * 2 * 128], bf16)
        b_t = [consts.tile([128, 512], bf16, name=f"bj{j}", tag=f"bj{j}")
               for j in range(JS)]
        e_sb = consts.tile([128, E_W], bf16)

        def in_col(c0, w):
            return in_d[:, c0:c0 + w]

        # IN column layout: [a0 256 | bj0 512 | bj1 512 | a123 768 |
        #                    bj2 512 | bj3 512 | e 160]
        nc.sync.dma_start(out=a0_sb, in_=in_col(0, 256))
        nc.sync.dma_start(out=b_t[0], in_=in_col(256, 512))
        nc.gpsimd.dma_start(out=b_t[1], in_=in_col(768, 512))
        nc.sync.dma_start(out=a123_sb, in_=in_col(1280, 768))
        nc.gpsimd.dma_start(out=b_t[2], in_=in_col(2048, 512))
        nc.sync.dma_start(out=b_t[3], in_=in_col(2560, 512))
        nc.gpsimd.dma_start(out=e_sb, in_=in_col(3072, E_W))

        # views: A[b] -> [128, 2, 128], E [128, B, NT, 5]
        a0_v = a0_sb.rearrange("p (h m) -> p h m", h=2)
        a123_v = a123_sb.rearrange("p (b h m) -> p b h m", b=3, h=2)
        e_v = e_sb.rearrange("p (b t c) -> p b t c", b=B, t=NT)

        def a_slice(b, row, h4):
            if b == 0:
                return a0_v[32 * row:32 * row + K, h4, :]
            return a123_v[32 * row:32 * row + K, b - 1, h4, :]

        def b_slice(j):
            return b_t[j]

        out_sb = outp.tile([128, B, 3, XCOLS], f32)
        grams = {}

        def emit_mm1_group(b, slots, ps):
            """slots: global slot ids (b-local, 0..31), j = s//8, nt = s%8"""
            for i, s in enumerate(slots):
                j = s // 8
                nt = s % 8
                row = nt % 4
                lhsT = a_slice(b, row, nt // 4)
                rhs = b_slice(j)[32 * row:32 * row + K, :]
                nc.tensor.matmul(ps[:, i, :], lhsT, rhs,
                                 start=True, stop=True,
                                 tile_position=(32 * row, 0))

        def emit_mm1_exp(b, h, sizes, tags):
            """one gram tile covering the half's 16 slots (j-outer)"""
            gram = gram_pool.tile([128, 16, 512], bf16, tag="gram",
                                  name=f"gram{b}{h}")
            grams[(b, h)] = gram
            s0 = 0
            for gsz, sel in zip(sizes, tags):
                pool = (mm1a_pool, mm1b_pool)[sel]
                cap = (4, 3)[sel]
                ps = pool.tile([128, cap, 512], f32, tag=f"t{sel}",
                               name=f"ps{sel}")
                emit_mm1_group(b, [16 * h + s0 + i for i in range(gsz)], ps)
                nc.scalar.activation(out=gram[:, s0:s0 + gsz, :],
                                     in_=ps[:, 0:gsz, :], func=EXP)
                s0 += gsz

        def emit_mm1_exp_perj(b, j, sizes, tags):
            """per-j gram tile (8 slots) — used for the final half so the
            tail mm2 only waits on its own j block"""
            gram = gram_pool.tile([128, 8, 512], bf16, tag="gramj",
                                  name=f"gramj{b}{j}")
            grams[("j", b, j)] = gram
            s0 = 0
            for gsz, sel in zip(sizes, tags):
                pool = (mm1a_pool, mm1b_pool)[sel]
                cap = (4, 3)[sel]
                ps = pool.tile([128, cap, 512], f32, tag=f"t{sel}",
                               name=f"ps{sel}")
                emit_mm1_group(b, [8 * j + s0 + i for i in range(gsz)], ps)
                nc.scalar.activation(out=gram[:, s0:s0 + gsz, :],
                                     in_=ps[:, 0:gsz, :], func=EXP)
                s0 += gsz

        def emit_mm2_j(b, j, gram, base):
            fm = grams[("fm", b)]
            for r in range(4):
                gsub = j * 4 + r
                for nt in range(NT):
                    nc.tensor.matmul(
                        fm[:, gsub, :],
                        gram[:, base + nt, r * 128:(r + 1) * 128],
                        e_v[:, b, nt, :],
                        start=(nt == 0),
                        stop=(nt == NT - 1),
                    )

        def emit_norm(b, sl, dma_engine):
            """normalize fm[:, sl, :] and emit ONE combined output DMA
            covering all 3 channels of this gsub range"""
            fm = grams[("fm", b)]
            w = sl.stop - sl.start
            fmc = small.tile([128, 8, 5], f32, tag="fmc")
            nc.vector.tensor_copy(fmc[:, 0:w, :], fm[:, sl, :])
            recip = small.tile([128, 8], f32, tag="recip")
            nc.vector.reciprocal(recip[:, 0:w], fmc[:, 0:w, 0])
            nc.vector.tensor_copy(out_sb[:, b, 0, sl], fmc[:, 0:w, 0])
            v1 = small.tile([128, 8], f32, tag="v1")
            nc.vector.tensor_add(v1[:, 0:w], fmc[:, 0:w, 1], fmc[:, 0:w, 3])
            nc.vector.tensor_mul(out_sb[:, b, 1, sl], v1[:, 0:w], recip[:, 0:w])
            v2 = small.tile([128, 8], f32, tag="v2")
            nc.vector.tensor_add(v2[:, 0:w], fmc[:, 0:w, 2], fmc[:, 0:w, 4])
            nc.vector.tensor_mul(out_sb[:, b, 2, sl], v2[:, 0:w], recip[:, 0:w])
            # dst iterated (y, c, x) to match the SBUF (partition, c, x) order
            dst = out_d[b, :, :, sl].rearrange("c y x -> y c x")
            dma_engine.dma_start(out=dst, in_=out_sb[:, b, :, sl])

        # pattern schedule: first half starts with a 2-slot group so the
        # first exp fires as early as possible; thereafter tags alternate
        # across the boundary so ACT never waits for a psum refill.
        half_patterns = [((2, 4, 3, 4, 3), (1, 0, 1, 0, 1))]
        for k in range(1, 7):
            if k % 2 == 1:
                half_patterns.append(((4, 3, 4, 3, 2), (0, 1, 0, 1, 0)))
            else:
                half_patterns.append(((3, 4, 3, 4, 2), (1, 0, 1, 0, 1)))

        # software pipeline: mm1/exp of (b) overlaps mm2/norm of (b-1)
        for b in range(B):
            fm_t = mm2_pool.tile([128, XCOLS, 5], f32, tag="fm")
            grams[("fm", b)] = fm_t
            if b < B - 1:
                emit_mm1_exp(b, 0, *half_patterns[2 * b])
                emit_mm1_exp(b, 1, *half_patterns[2 * b + 1])
            else:
                emit_mm1_exp(b, 0, *half_patterns[6])
                emit_mm1_exp(b, 1, ((4, 3, 4, 3, 2)), ((0, 1, 0, 1, 0)))
            if b >= 1:
                p = b - 1
                for h in range(2):
                    g = grams[(p, h)]
                    emit_mm2_j(p, 2 * h, g, 0)
                    emit_mm2_j(p, 2 * h + 1, g, 8)
                    emit_norm(p, slice(8 * h, 8 * h + 8), nc.sync)
        b = B - 1
        for h in range(2):
            g = grams[(b, h)]
            emit_mm2_j(b, 2 * h, g, 0)
            emit_mm2_j(b, 2 * h + 1, g, 8)
            emit_norm(b, slice(8 * h, 8 * h + 8), nc.sync)

    nc.compile()
    return nc


# ---------------------------------------------------------------------------
# Separable path: the reference grid is a meshgrid, so the 2-D RBF factors
# into 1-D Gaussians:  Gram[(x,y), n] = A1[x, n] * A2[y, n].
#   FM[y, (x,c)] = sum_n A2[n, y] * (A1[x, n] * E[n, c])
# which needs exp on only ~0.6M elements/core instead of 8.4M.
# ---------------------------------------------------------------------------

KS = 7                     # 1-D factorization rows
S_W = B * NT * 128         # stationary block width (per X component)
INS_W = 2 * S_W + 128 + XCOLS   # S2 | S1 | G2y | G1x


def _build_nc_sep():
    import concourse.bacc as bacc
    import concourse.mybir as mybir
    import concourse.tile as tile
    from contextlib import ExitStack

    f32 = mybir.dt.float32
    f16 = mybir.dt.float16

    nc = bacc.Bacc("TRN2", target_bir_lowering=False, debug=False,
                   num_devices=NCORES)
    ins_d = nc.dram_tensor("INS", [KS, INS_W], f16, kind="ExternalInput")
    ine_d = nc.dram_tensor("INE", [128, B * NT * 3], f32, kind="ExternalInput")
    out_d = nc.dram_tensor("OUT", [B, 3, 128, XCOLS], f32, kind="ExternalOutput")

    EXP = mybir.ActivationFunctionType.Exp

    with tile.TileContext(nc) as tc, ExitStack() as ctx:
        consts = ctx.enter_context(tc.tile_pool(name="consts", bufs=1))
        a2_pool = ctx.enter_context(tc.tile_pool(name="a2", bufs=2))
        a1_pool = ctx.enter_context(tc.tile_pool(name="a1", bufs=2))
        w_pool = ctx.enter_context(tc.tile_pool(name="w", bufs=2))
        ps2_pool = ctx.enter_context(tc.tile_pool(name="ps2", bufs=2, space="PSUM"))
        ps1_pool = ctx.enter_context(tc.tile_pool(name="ps1", bufs=2, space="PSUM"))
        fm_pool = ctx.enter_context(tc.tile_pool(name="fmp", bufs=2, space="PSUM"))
        small = ctx.enter_context(tc.tile_pool(name="small", bufs=4))
        outp = ctx.enter_context(tc.tile_pool(name="outp", bufs=1))

        ins_sb = consts.tile([KS, INS_W], f16)
        e_sb = consts.tile([128, B * NT * 3], f32)
        nc.sync.dma_start(out=ins_sb, in_=ins_d[:])
        nc.gpsimd.dma_start(out=e_sb, in_=ine_d[:])

        s2_v = ins_sb[:, 0:S_W].rearrange("k (b t m) -> k b t m", b=B, t=NT)
        s1_v = ins_sb[:, S_W:2 * S_W].rearrange("k (b t m) -> k b t m", b=B, t=NT)
        g2_v = ins_sb[:, 2 * S_W:2 * S_W + 128]
        g1_v = ins_sb[:, 2 * S_W + 128:INS_W]
        e_v = e_sb.rearrange("p (b t c) -> p b t c", b=B, t=NT)

        out_sb = outp.tile([128, B, 3, XCOLS], f32)
        st = {}

        def emit_b(b):
            # A2 = exp(-0.5 (gy - X1)^2): [n(128), 8nt, y(128)] fp16
            a2 = a2_pool.tile([128, NT, 128], f16, tag="a2", name=f"a2_{b}")
            for g in range(2):
                ps2 = ps2_pool.tile([128, 4, 128], f32, tag="ps2", name="ps2")
                for i in range(4):
                    nt = 4 * g + i
                    nc.tensor.matmul(ps2[:, i, :], s2_v[:, b, nt, :], g2_v,
                                     start=True, stop=True)
                nc.scalar.activation(out=a2[:, 4 * g:4 * g + 4, :], in_=ps2,
                                     func=EXP)
            # A1 = exp(-0.5 (gx - X0)^2): [n(128), 8nt, x(16)] fp16
            a1 = a1_pool.tile([128, NT, XCOLS], f16, tag="a1", name=f"a1_{b}")
            ps1 = ps1_pool.tile([128, NT, XCOLS], f32, tag="ps1", name="ps1")
            for nt in range(NT):
                nc.tensor.matmul(ps1[:, nt, :], s1_v[:, b, nt, :], g1_v,
                                 start=True, stop=True)
            nc.scalar.activation(out=a1, in_=ps1, func=EXP)
            # W[n, nt, x, c] = A1[n, nt, x] * E[n, nt, c]
            w = w_pool.tile([128, NT, XCOLS, 3], f16, tag="w", name=f"w_{b}")
            for nt in range(NT):
                for c in range(3):
                    nc.vector.tensor_scalar_mul(
                        w[:, nt, :, c], a1[:, nt, :], e_v[:, b, nt, c:c + 1])
            # FM[y, x, c] = sum_nt A2[:, nt, :].T @ W[:, nt, :, :]
            fm = fm_pool.tile([128, XCOLS, 3], f32, tag="fm", name=f"fm_{b}")
            st[b] = fm
            for nt in range(NT):
                nc.tensor.matmul(fm[:], a2[:, nt, :], w[:, nt, :, :],
                                 start=(nt == 0), stop=(nt == NT - 1))

        def emit_norm(b):
            fm = st[b]
            fmc = small.tile([128, XCOLS, 3], f32, tag="fmc")
            nc.vector.tensor_copy(fmc, fm)
            recip = small.tile([128, XCOLS], f32, tag="recip")
            nc.vector.reciprocal(recip, fmc[:, :, 0])
            nc.vector.tensor_copy(out_sb[:, b, 0, :], fmc[:, :, 0])
            nc.vector.tensor_mul(out_sb[:, b, 1, :], fmc[:, :, 1], recip)
            nc.vector.tensor_mul(out_sb[:, b, 2, :], fmc[:, :, 2], recip)
            dst = out_d[b].rearrange("c y x -> y c x")
            nc.sync.dma_start(out=dst, in_=out_sb[:, b, :, :])

        for b in range(B):
            emit_b(b)
            if b >= 1:
                emit_norm(b - 1)
        emit_norm(B - 1)

    nc.compile()
    return nc


def _sep_factors(gv, xc):
    """K=7 fp16 factorization of -0.5 (g - x)^2 along one dimension.
    Returns A (B, 7, N) stationary rows and Bm (7, len(gv)) moving rows."""
    f16 = np.float16

    def split(a):
        hi = a.astype(f16).astype(np.float32)
        lo = (a - hi).astype(f16).astype(np.float32)
        return hi, lo

    sx = -0.5 * xc * xc
    sg = -0.5 * gv * gv
    xh, xl = split(xc)
    gh, gl = split(gv)
    sxh, sxl = split(sx)
    sgh, sgl = split(sg)
    on = np.ones_like(xc)
    og = np.ones_like(gv)
    A = np.stack([xh, xl, xh, sxh, sxl, on, on], axis=1)
    Bm = np.stack([gh, gh, gl, og, og, sgh, sgl], axis=0)
    return A.astype(f16), Bm.astype(f16)


def _prepare_inputs_sep(X, Y, grid):
    f16 = np.float16
    X = np.asarray(X, np.float32)
    Y = np.asarray(Y, np.float32)
    grid = np.asarray(grid, np.float32)
    gxv = grid[::128, 0]
    gyv = grid[:128, 1]

    A2s, G2 = _sep_factors(gyv, X[..., 1])   # (B,7,N), (7,128)
    A1s, G1 = _sep_factors(gxv, X[..., 0])   # (B,7,N), (7,128x_all)
    # stationary blocks [7, B, NT, 128]
    s2 = A2s.transpose(1, 0, 2).reshape(KS, B, NT, 128)
    s1 = A1s.transpose(1, 0, 2).reshape(KS, B, NT, 128)
    E = np.concatenate([np.ones((B, N, 1), np.float32), Y], axis=-1)
    ine = np.ascontiguousarray(
        E.reshape(B, NT, 128, 3).transpose(2, 0, 1, 3).reshape(128, B * NT * 3))

    in_maps = []
    s2f = s2.reshape(KS, S_W)
    s1f = s1.reshape(KS, S_W)
    for c in range(NCORES):
        g1c = G1[:, c * XCOLS:(c + 1) * XCOLS]
        ins = np.concatenate([s2f, s1f, G2, g1c], axis=1).astype(f16)
        in_maps.append({"INS": np.ascontiguousarray(ins), "INE": ine})
    return in_maps


def _grid_separable(grid):
    grid = np.asarray(grid)
    if grid.shape != (G, 2):
        return False
    gxv = grid[::128, 0]
    gyv = grid[:128, 1]
    return (np.array_equal(grid[:, 0], np.repeat(gxv, 128))
            and np.array_equal(grid[:, 1], np.tile(gyv, 128)))


def _split_hi_lo(a):
    import ml_dtypes

    bf = ml_dtypes.bfloat16
    hi = a.astype(bf).astype(np.float32)
    lo = (a - hi).astype(bf).astype(np.float32)
    return hi, lo


def _prepare_inputs(X, Y, grid):
    """Host-side packing: per-core input maps for the SPMD kernel."""
    import ml_dtypes

    bf = ml_dtypes.bfloat16
    X = np.asarray(X, np.float32)
    Y = np.asarray(Y, np.float32)
    grid = np.asarray(grid, np.float32)

    sx = -0.5 * np.sum(X * X, axis=-1)        # (B, N)
    sg = -0.5 * np.sum(grid * grid, axis=-1)  # (G,)
    xh, xl = _split_hi_lo(X)
    gh, gl = _split_hi_lo(grid)
    sxh, sxl = _split_hi_lo(sx)
    sgh, sgl = _split_hi_lo(sg)
    ones_n = np.ones((B, N), np.float32)
    ones_g = np.ones((G,), np.float32)

    # M'[n, g] = sum_k A[k, n] * Bm[k, g] = g.x - 0.5|x|^2 - 0.5|g|^2
    A = np.stack(
        [xh[..., 0], xh[..., 1], xl[..., 0], xl[..., 1],
         xh[..., 0], xh[..., 1], sxh, sxl, ones_n, ones_n],
        axis=1,
    )  # (B, K, N)
    Bm = np.stack(
        [gh[:, 0], gh[:, 1], gh[:, 0], gh[:, 1],
         gl[:, 0], gl[:, 1], ones_g, ones_g, sgh, sgl],
        axis=0,
    )  # (K, G)

    # A replicated into 4 row-strips: strip i (partitions 32i..32i+9)
    # holds A rows for nt = h*4 + i  -> [128, B, 2, 128]
    A4 = A.transpose(1, 0, 2).reshape(K, B, 2, 4, 128)  # k b h i p
    arep = np.zeros((128, B, 2, 128), np.float32)
    for i in range(4):
        arep[32 * i:32 * i + K] = A4[:, :, :, i, :]

    # E: [128, B, NT, 5] = [1, yh0, yh1, yl0, yl1]
    yh, yl = _split_hi_lo(Y)
    E = np.stack([ones_n, yh[..., 0], yh[..., 1], yl[..., 0], yl[..., 1]],
                 axis=-1)
    ey = E.reshape(B, NT, 128, 5).transpose(2, 0, 1, 3)

    in_maps = []
    ar = arep.reshape(128, A_W)
    for c in range(NCORES):
        # B replicated into the same 4 row-strips
        brep = np.zeros((128, GS), np.float32)
        for i in range(4):
            brep[32 * i:32 * i + K] = Bm[:, c * GS:(c + 1) * GS]
        # column order: [a0 | bj0 | bj1 | a123 | bj2 | bj3 | e]
        packed = np.concatenate(
            [ar[:, 0:256], brep[:, 0:512], brep[:, 512:1024],
             ar[:, 256:A_W], brep[:, 1024:1536], brep[:, 1536:2048],
             ey.reshape(128, E_W)], axis=1)
        in_maps.append({"IN": np.ascontiguousarray(packed).astype(bf)})
    return in_maps


def _run(in_maps, builder, key, trace=False):
    from concourse.bass_utils import run_bass_kernel_spmd

    if key not in _CACHE:
        _CACHE[key] = builder()
    nc = _CACHE[key]
    return run_bass_kernel_spmd(nc, in_maps, core_ids=list(range(NCORES)),
                                trace=trace)


def kernel(X, Y, grid, _trace=False, _results_out=None):
    if _grid_separable(grid):
        in_maps = _prepare_inputs_sep(X, Y, grid)
        res = _run(in_maps, _build_nc_sep, "sep", trace=_trace)
    else:
        in_maps = _prepare_inputs(X, Y, grid)
        res = _run(in_maps, _build_nc_general, "gen", trace=_trace)
    out = np.empty((B, 3, 128, 128), np.float32)
    for c in range(NCORES):
        out[:, :, :, c * XCOLS:(c + 1) * XCOLS] = res.results[c]["OUT"]
    if _results_out is not None:
        _results_out.append(res)
    return out


# revision 23
# speedup vs baseline: 3.1985x; 1.3930x over previous
"""ConvCNP encoder kernel for 8 Trainium2 NeuronCores.

Computes, for full inputs X(4,1024,2), Y(4,1024,2), grid(16384,2):
    Gram = exp(-0.5*||grid-X||^2)          (B, G, n)
    FM   = Gram @ [1, Y]                   (B, G, 3)
    out  = [FM0, FM1/FM0, FM2/FM0] -> (B, 3, 128, 128)  (y, x image axes)

Sharding: grid axis G split 8 ways (2048 rows / core = 16 output
x-columns); every core handles all 4 batches; no cross-device reduction.

Per-core device pipeline:
  mm1 (PE):  -0.5*d2 as K=10 bf16 matmuls using a hi/lo split of
             g.x - 0.5|g|^2 - 0.5|x|^2 -> PSUM [n-tile 128, g 512].
             K=10 << 128, so 4 matmuls run concurrently in 32-row
             strips of the PE array via tile_position (inputs are
             replicated at partition offsets 0/32/64/96).
  exp (ACT): PSUM -> SBUF Gram (bf16), batched [128, <=4*512] exps
  mm2 (PE):  Gram[n,g128] stationary x E[n, (1,Yhi,Ylo)] moving,
             accumulated over 8 n-tiles -> FM [g(=y) 128, 5] PSUM
  norm (DVE): density reciprocal + multiplies, fp32
  DMA out:   [y, x] tiles per (b, c)
"""

import numpy as np

B = 4
N = 1024
G = 16384
NCORES = 8
GS = G // NCORES          # 2048 grid rows per core
NT = N // 128             # 8 context tiles
JS = GS // 512            # 4 g-blocks of 512 per core
K = 10                    # contraction rows of the d2 factorization
XCOLS = GS // 128         # 16 output x-columns per core

# combined input layout (free-dim offsets, bf16 elements)
A_W = B * 2 * 128         # 1024
B_W = GS                  # 2048
E_W = B * NT * 5          # 160
IN_W = A_W + B_W + E_W    # 3232

_CACHE = {}


def _build_nc_general():
    import concourse.bacc as bacc
    import concourse.mybir as mybir
    import concourse.tile as tile
    from contextlib import ExitStack

    f32 = mybir.dt.float32
    bf16 = mybir.dt.bfloat16

    nc = bacc.Bacc("TRN2", target_bir_lowering=False, debug=False,
                   num_devices=NCORES)
    in_d = nc.dram_tensor("IN", [128, IN_W], bf16, kind="ExternalInput")
    out_d = nc.dram_tensor("OUT", [B, 3, 128, XCOLS], f32, kind="ExternalOutput")

    EXP = mybir.ActivationFunctionType.Exp

    with tile.TileContext(nc) as tc, ExitStack() as ctx:
        consts = ctx.enter_context(tc.tile_pool(name="consts", bufs=1))
        gram_pool = ctx.enter_context(tc.tile_pool(name="gram", bufs=4))
        mm1a_pool = ctx.enter_context(tc.tile_pool(name="mm1a", bufs=1, space="PSUM"))
        mm1b_pool = ctx.enter_context(tc.tile_pool(name="mm1b", bufs=1, space="PSUM"))
        mm2_pool = ctx.enter_context(tc.tile_pool(name="mm2", bufs=1, space="PSUM"))
        small = ctx.enter_context(tc.tile_pool(name="small", bufs=4))
        outp = ctx.enter_context(tc.tile_pool(name="outp", bufs=1))

        # separate tiles so consumers only wait for the DMA they need;
        # the first mm1 group touches only A[b0] and B[j0] — land those
        # first, one on each queue, before the bulk.
        a0_sb = consts.tile([128, 2 * 128], bf16)
        a123_sb = consts.tile([128, 3 * 2 * 128], bf16)
        b_t = [consts.tile([128, 512], bf16, name=f"bj{j}", tag=f"bj{j}")
               for j in range(JS)]
        e_sb = consts.tile([128, E_W], bf16)

        def in_col(c0, w):
            return in_d[:, c0:c0 + w]

        # IN column layout: [a0 256 | bj0 512 | bj1 512 | a123 768 |
        #                    bj2 512 | bj3 512 | e 160]
        nc.sync.dma_start(out=a0_sb, in_=in_col(0, 256))
        nc.sync.dma_start(out=b_t[0], in_=in_col(256, 512))
        nc.gpsimd.dma_start(out=b_t[1], in_=in_col(768, 512))
        nc.sync.dma_start(out=a123_sb, in_=in_col(1280, 768))
        nc.gpsimd.dma_start(out=b_t[2], in_=in_col(2048, 512))
        nc.sync.dma_start(out=b_t[3], in_=in_col(2560, 512))
        nc.gpsimd.dma_start(out=e_sb, in_=in_col(3072, E_W))

        # views: A[b] -> [128, 2, 128], E [128, B, NT, 5]
        a0_v = a0_sb.rearrange("p (h m) -> p h m", h=2)
        a123_v = a123_sb.rearrange("p (b h m) -> p b h m", b=3, h=2)
        e_v = e_sb.rearrange("p (b t c) -> p b t c", b=B, t=NT)

        def a_slice(b, row, h4):
            if b == 0:
                return a0_v[32 * row:32 * row + K, h4, :]
            return a123_v[32 * row:32 * row + K, b - 1, h4, :]

        def b_slice(j):
            return b_t[j]

        out_sb = outp.tile([128, B, 3, XCOLS], f32)
        grams = {}

        def emit_mm1_group(b, slots, ps):
            """slots: global slot ids (b-local, 0..31), j = s//8, nt = s%8"""
            for i, s in enumerate(slots):
                j = s // 8
                nt = s % 8
                row = nt % 4
                lhsT = a_slice(b, row, nt // 4)
                rhs = b_slice(j)[32 * row:32 * row + K, :]
                nc.tensor.matmul(ps[:, i, :], lhsT, rhs,
                                 start=True, stop=True,
                                 tile_position=(32 * row, 0))

        def emit_mm1_exp(b, h, sizes, tags):
            """one gram tile covering the half's 16 slots (j-outer)"""
            gram = gram_pool.tile([128, 16, 512], bf16, tag="gram",
                                  name=f"gram{b}{h}")
            grams[(b, h)] = gram
            s0 = 0
            for gsz, sel in zip(sizes, tags):
                pool = (mm1a_pool, mm1b_pool)[sel]
                cap = (4, 3)[sel]
                ps = pool.tile([128, cap, 512], f32, tag=f"t{sel}",
                               name=f"ps{sel}")
                emit_mm1_group(b, [16 * h + s0 + i for i in range(gsz)], ps)
                nc.scalar.activation(out=gram[:, s0:s0 + gsz, :],
                                     in_=ps[:, 0:gsz, :], func=EXP)
                s0 += gsz

        def emit_mm1_exp_perj(b, j, sizes, tags):
            """per-j gram tile (8 slots) — used for the final half so the
            tail mm2 only waits on its own j block"""
            gram = gram_pool.tile([128, 8, 512], bf16, tag="gramj",
                                  name=f"gramj{b}{j}")
            grams[("j", b, j)] = gram
            s0 = 0
            for gsz, sel in zip(sizes, tags):
                pool = (mm1a_pool, mm1b_pool)[sel]
                cap = (4, 3)[sel]
                ps = pool.tile([128, cap, 512], f32, tag=f"t{sel}",
                               name=f"ps{sel}")
                emit_mm1_group(b, [8 * j + s0 + i for i in range(gsz)], ps)
                nc.scalar.activation(out=gram[:, s0:s0 + gsz, :],
                                     in_=ps[:, 0:gsz, :], func=EXP)
                s0 += gsz

        def emit_mm2_j(b, j, gram, base):
            fm = grams[("fm", b)]
            for r in range(4):
                gsub = j * 4 + r
                for nt in range(NT):
                    nc.tensor.matmul(
                        fm[:, gsub, :],
                        gram[:, base + nt, r * 128:(r + 1) * 128],
                        e_v[:, b, nt, :],
                        start=(nt == 0),
                        stop=(nt == NT - 1),
                    )

        def emit_norm(b, sl, dma_engine):
            """normalize fm[:, sl, :] and emit ONE combined output DMA
            covering all 3 channels of this gsub range"""
            fm = grams[("fm", b)]
            w = sl.stop - sl.start
            fmc = small.tile([128, 8, 5], f32, tag="fmc")
            nc.vector.tensor_copy(fmc[:, 0:w, :], fm[:, sl, :])
            recip = small.tile([128, 8], f32, tag="recip")
            nc.vector.reciprocal(recip[:, 0:w], fmc[:, 0:w, 0])
            nc.vector.tensor_copy(out_sb[:, b, 0, sl], fmc[:, 0:w, 0])
            v1 = small.tile([128, 8], f32, tag="v1")
            nc.vector.tensor_add(v1[:, 0:w], fmc[:, 0:w, 1], fmc[:, 0:w, 3])
            nc.vector.tensor_mul(out_sb[:, b, 1, sl], v1[:, 0:w], recip[:, 0:w])
            v2 = small.tile([128, 8], f32, tag="v2")
            nc.vector.tensor_add(v2[:, 0:w], fmc[:, 0:w, 2], fmc[:, 0:w, 4])
            nc.vector.tensor_mul(out_sb[:, b, 2, sl], v2[:, 0:w], recip[:, 0:w])
            # dst iterated (y, c, x) to match the SBUF (partition, c, x) order
            dst = out_d[b, :, :, sl].rearrange("c y x -> y c x")
            dma_engine.dma_start(out=dst, in_=out_sb[:, b, :, sl])

        # pattern schedule: first half starts with a 2-slot group so the
        # first exp fires as early as possible; thereafter tags alternate
        # across the boundary so ACT never waits for a psum refill.
        half_patterns = [((2, 4, 3, 4, 3), (1, 0, 1, 0, 1))]
        for k in range(1, 7):
            if k % 2 == 1:
                half_patterns.append(((4, 3, 4, 3, 2), (0, 1, 0, 1, 0)))
            else:
                half_patterns.append(((3, 4, 3, 4, 2), (1, 0, 1, 0, 1)))

        # software pipeline: mm1/exp of (b) overlaps mm2/norm of (b-1)
        for b in range(B):
            fm_t = mm2_pool.tile([128, XCOLS, 5], f32, tag="fm")
            grams[("fm", b)] = fm_t
            if b < B - 1:
                emit_mm1_exp(b, 0, *half_patterns[2 * b])
                emit_mm1_exp(b, 1, *half_patterns[2 * b + 1])
            else:
                emit_mm1_exp(b, 0, *half_patterns[6])
                emit_mm1_exp(b, 1, ((4, 3, 4, 3, 2)), ((0, 1, 0, 1, 0)))
            if b >= 1:
                p = b - 1
                for h in range(2):
                    g = grams[(p, h)]
                    emit_mm2_j(p, 2 * h, g, 0)
                    emit_mm2_j(p, 2 * h + 1, g, 8)
                    emit_norm(p, slice(8 * h, 8 * h + 8), nc.sync)
        b = B - 1
        for h in range(2):
            g = grams[(b, h)]
            emit_mm2_j(b, 2 * h, g, 0)
            emit_mm2_j(b, 2 * h + 1, g, 8)
            emit_norm(b, slice(8 * h, 8 * h + 8), nc.sync)

    nc.compile()
    return nc


# ---------------------------------------------------------------------------
# Separable path: the reference grid is a meshgrid, so the 2-D RBF factors
# into 1-D Gaussians:  Gram[(x,y), n] = A1[x, n] * A2[y, n].
#   FM[y, (x,c)] = sum_n A2[n, y] * (A1[x, n] * E[n, c])
# which needs exp on only ~0.6M elements/core instead of 8.4M.
# ---------------------------------------------------------------------------

KS = 7                     # 1-D factorization rows
S_W = B * NT * 128         # stationary block width (per X component)
INS_W = 2 * S_W + 128 + XCOLS   # G2y | G1x | S2 | S1


def _build_nc_sep():
    import concourse.bacc as bacc
    import concourse.mybir as mybir
    import concourse.tile as tile
    from contextlib import ExitStack

    f32 = mybir.dt.float32
    f16 = mybir.dt.float16

    nc = bacc.Bacc("TRN2", target_bir_lowering=False, debug=False,
                   num_devices=NCORES)
    ins_d = nc.dram_tensor("INS", [KS, INS_W], f16, kind="ExternalInput")
    ine_d = nc.dram_tensor("INE", [128, B * NT * 3], f32, kind="ExternalInput")
    out_d = nc.dram_tensor("OUT", [B, 3, 128, XCOLS], f32, kind="ExternalOutput")

    EXP = mybir.ActivationFunctionType.Exp

    with tile.TileContext(nc) as tc, ExitStack() as ctx:
        consts = ctx.enter_context(tc.tile_pool(name="consts", bufs=1))
        a2_pool = ctx.enter_context(tc.tile_pool(name="a2", bufs=2))
        a1_pool = ctx.enter_context(tc.tile_pool(name="a1", bufs=2))
        w_pool = ctx.enter_context(tc.tile_pool(name="w", bufs=2))
        ps2_pool = ctx.enter_context(tc.tile_pool(name="ps2", bufs=2, space="PSUM"))
        ps1_pool = ctx.enter_context(tc.tile_pool(name="ps1", bufs=2, space="PSUM"))
        fm_pool = ctx.enter_context(tc.tile_pool(name="fmp", bufs=2, space="PSUM"))
        small = ctx.enter_context(tc.tile_pool(name="small", bufs=4))
        outp = ctx.enter_context(tc.tile_pool(name="outp", bufs=1))

        ins_sb = consts.tile([KS, INS_W], f16)
        s1_sb = consts.tile([KS, S_W], f16, name="s1_sb")
        e_sb = consts.tile([128, B * NT * 3], f32)
        # column layout: [G2 128 | G1 16 | S2]; S1 and E ride gpsimd
        nc.sync.dma_start(out=ins_sb[:, 0:144 + S_W],
                          in_=ins_d[:, 0:144 + S_W])
        nc.gpsimd.dma_start(out=s1_sb, in_=ins_d[:, 144 + S_W:INS_W])
        nc.gpsimd.dma_start(out=e_sb, in_=ine_d[:])

        g2_v = ins_sb[:, 0:128]
        g1_v = ins_sb[:, 128:144]
        s2_v = ins_sb[:, 144:144 + S_W].rearrange("k (b t m) -> k b t m",
                                                  b=B, t=NT)
        s1_v = s1_sb.rearrange("k (b t m) -> k b t m", b=B, t=NT)
        e_v = e_sb.rearrange("p (b t c) -> p b t c", b=B, t=NT)

        out_sb = outp.tile([128, B, 3, XCOLS], f32)
        st = {}

        def emit_b(b):
            # A2 = exp(-0.5 (gy - X1)^2): [n(128), 8nt, y(128)] fp16
            a2 = a2_pool.tile([128, NT, 128], f16, tag="a2", name=f"a2_{b}")
            for g in range(2):
                ps2 = ps2_pool.tile([128, 4, 128], f32, tag="ps2", name="ps2")
                for i in range(4):
                    nt = 4 * g + i
                    nc.tensor.matmul(ps2[:, i, :], s2_v[:, b, nt, :], g2_v,
                                     start=True, stop=True)
                nc.scalar.activation(out=a2[:, 4 * g:4 * g + 4, :], in_=ps2,
                                     func=EXP)
            # A1 = exp(-0.5 (gx - X0)^2): [n(128), 8nt, x(16)] fp16
            a1 = a1_pool.tile([128, NT, XCOLS], f16, tag="a1", name=f"a1_{b}")
            ps1 = ps1_pool.tile([128, NT, XCOLS], f32, tag="ps1", name="ps1")
            for nt in range(NT):
                nc.tensor.matmul(ps1[:, nt, :], s1_v[:, b, nt, :], g1_v,
                                 start=True, stop=True)
            nc.scalar.activation(out=a1, in_=ps1, func=EXP)
            # W[n, nt, x, c] = A1[n, nt, x] * E[n, nt, c] — one broadcast
            # multiply over the whole batch
            w = w_pool.tile([128, NT, XCOLS, 3], f16, tag="w", name=f"w_{b}")
            a1_b = a1[:, :, :, None].to_broadcast([128, NT, XCOLS, 3])
            e_b = e_v[:, b, :, None, :].to_broadcast([128, NT, XCOLS, 3])
            nc.vector.tensor_mul(w, a1_b, e_b)
            # FM[y, x, c] = sum_nt A2[:, nt, :].T @ W[:, nt, :, :]
            fm = fm_pool.tile([128, XCOLS, 3], f32, tag="fm", name=f"fm_{b}")
            st[b] = fm
            for nt in range(NT):
                nc.tensor.matmul(fm[:], a2[:, nt, :], w[:, nt, :, :],
                                 start=(nt == 0), stop=(nt == NT - 1))

        def emit_norm(b):
            fm = st[b]
            fmc = small.tile([128, XCOLS, 3], f32, tag="fmc")
            nc.vector.tensor_copy(fmc, fm)
            recip = small.tile([128, XCOLS], f32, tag="recip")
            nc.vector.reciprocal(recip, fmc[:, :, 0])
            nc.vector.tensor_copy(out_sb[:, b, 0, :], fmc[:, :, 0])
            nc.vector.tensor_mul(out_sb[:, b, 1, :], fmc[:, :, 1], recip)
            nc.vector.tensor_mul(out_sb[:, b, 2, :], fmc[:, :, 2], recip)
            dst = out_d[b].rearrange("c y x -> y c x")
            nc.sync.dma_start(out=dst, in_=out_sb[:, b, :, :])

        for b in range(B):
            emit_b(b)
            if b >= 1:
                emit_norm(b - 1)
        emit_norm(B - 1)

    nc.compile()
    return nc


def _sep_factors(gv, xc):
    """K=7 fp16 factorization of -0.5 (g - x)^2 along one dimension.
    Returns A (B, 7, N) stationary rows and Bm (7, len(gv)) moving rows."""
    f16 = np.float16

    def split(a):
        hi = a.astype(f16).astype(np.float32)
        lo = (a - hi).astype(f16).astype(np.float32)
        return hi, lo

    sx = -0.5 * xc * xc
    sg = -0.5 * gv * gv
    xh, xl = split(xc)
    gh, gl = split(gv)
    sxh, sxl = split(sx)
    sgh, sgl = split(sg)
    on = np.ones_like(xc)
    og = np.ones_like(gv)
    A = np.stack([xh, xl, xh, sxh, sxl, on, on], axis=1)
    Bm = np.stack([gh, gh, gl, og, og, sgh, sgl], axis=0)
    return A.astype(f16), Bm.astype(f16)


def _prepare_inputs_sep(X, Y, grid):
    f16 = np.float16
    X = np.asarray(X, np.float32)
    Y = np.asarray(Y, np.float32)
    grid = np.asarray(grid, np.float32)
    gxv = grid[::128, 0]
    gyv = grid[:128, 1]

    A2s, G2 = _sep_factors(gyv, X[..., 1])   # (B,7,N), (7,128)
    A1s, G1 = _sep_factors(gxv, X[..., 0])   # (B,7,N), (7,128x_all)
    # stationary blocks [7, B, NT, 128]
    s2 = A2s.transpose(1, 0, 2).reshape(KS, B, NT, 128)
    s1 = A1s.transpose(1, 0, 2).reshape(KS, B, NT, 128)
    E = np.concatenate([np.ones((B, N, 1), np.float32), Y], axis=-1)
    ine = np.ascontiguousarray(
        E.reshape(B, NT, 128, 3).transpose(2, 0, 1, 3).reshape(128, B * NT * 3))

    in_maps = []
    s2f = s2.reshape(KS, S_W)
    s1f = s1.reshape(KS, S_W)
    for c in range(NCORES):
        g1c = G1[:, c * XCOLS:(c + 1) * XCOLS]
        ins = np.concatenate([G2, g1c, s2f, s1f], axis=1).astype(f16)
        in_maps.append({"INS": np.ascontiguousarray(ins), "INE": ine})
    return in_maps


def _grid_separable(grid):
    grid = np.asarray(grid)
    if grid.shape != (G, 2):
        return False
    gxv = grid[::128, 0]
    gyv = grid[:128, 1]
    return (np.array_equal(grid[:, 0], np.repeat(gxv, 128))
            and np.array_equal(grid[:, 1], np.tile(gyv, 128)))


def _split_hi_lo(a):
    import ml_dtypes

    bf = ml_dtypes.bfloat16
    hi = a.astype(bf).astype(np.float32)
    lo = (a - hi).astype(bf).astype(np.float32)
    return hi, lo


def _prepare_inputs(X, Y, grid):
    """Host-side packing: per-core input maps for the SPMD kernel."""
    import ml_dtypes

    bf = ml_dtypes.bfloat16
    X = np.asarray(X, np.float32)
    Y = np.asarray(Y, np.float32)
    grid = np.asarray(grid, np.float32)

    sx = -0.5 * np.sum(X * X, axis=-1)        # (B, N)
    sg = -0.5 * np.sum(grid * grid, axis=-1)  # (G,)
    xh, xl = _split_hi_lo(X)
    gh, gl = _split_hi_lo(grid)
    sxh, sxl = _split_hi_lo(sx)
    sgh, sgl = _split_hi_lo(sg)
    ones_n = np.ones((B, N), np.float32)
    ones_g = np.ones((G,), np.float32)

    # M'[n, g] = sum_k A[k, n] * Bm[k, g] = g.x - 0.5|x|^2 - 0.5|g|^2
    A = np.stack(
        [xh[..., 0], xh[..., 1], xl[..., 0], xl[..., 1],
         xh[..., 0], xh[..., 1], sxh, sxl, ones_n, ones_n],
        axis=1,
    )  # (B, K, N)
    Bm = np.stack(
        [gh[:, 0], gh[:, 1], gh[:, 0], gh[:, 1],
         gl[:, 0], gl[:, 1], ones_g, ones_g, sgh, sgl],
        axis=0,
    )  # (K, G)

    # A replicated into 4 row-strips: strip i (partitions 32i..32i+9)
    # holds A rows for nt = h*4 + i  -> [128, B, 2, 128]
    A4 = A.transpose(1, 0, 2).reshape(K, B, 2, 4, 128)  # k b h i p
    arep = np.zeros((128, B, 2, 128), np.float32)
    for i in range(4):
        arep[32 * i:32 * i + K] = A4[:, :, :, i, :]

    # E: [128, B, NT, 5] = [1, yh0, yh1, yl0, yl1]
    yh, yl = _split_hi_lo(Y)
    E = np.stack([ones_n, yh[..., 0], yh[..., 1], yl[..., 0], yl[..., 1]],
                 axis=-1)
    ey = E.reshape(B, NT, 128, 5).transpose(2, 0, 1, 3)

    in_maps = []
    ar = arep.reshape(128, A_W)
    for c in range(NCORES):
        # B replicated into the same 4 row-strips
        brep = np.zeros((128, GS), np.float32)
        for i in range(4):
            brep[32 * i:32 * i + K] = Bm[:, c * GS:(c + 1) * GS]
        # column order: [a0 | bj0 | bj1 | a123 | bj2 | bj3 | e]
        packed = np.concatenate(
            [ar[:, 0:256], brep[:, 0:512], brep[:, 512:1024],
             ar[:, 256:A_W], brep[:, 1024:1536], brep[:, 1536:2048],
             ey.reshape(128, E_W)], axis=1)
        in_maps.append({"IN": np.ascontiguousarray(packed).astype(bf)})
    return in_maps


def _run(in_maps, builder, key, trace=False):
    from concourse.bass_utils import run_bass_kernel_spmd

    if key not in _CACHE:
        _CACHE[key] = builder()
    nc = _CACHE[key]
    return run_bass_kernel_spmd(nc, in_maps, core_ids=list(range(NCORES)),
                                trace=trace)


def kernel(X, Y, grid, _trace=False, _results_out=None):
    if _grid_separable(grid):
        in_maps = _prepare_inputs_sep(X, Y, grid)
        res = _run(in_maps, _build_nc_sep, "sep", trace=_trace)
    else:
        in_maps = _prepare_inputs(X, Y, grid)
        res = _run(in_maps, _build_nc_general, "gen", trace=_trace)
    out = np.empty((B, 3, 128, 128), np.float32)
    for c in range(NCORES):
        out[:, :, :, c * XCOLS:(c + 1) * XCOLS] = res.results[c]["OUT"]
    if _results_out is not None:
        _results_out.append(res)
    return out


# revision 24
# speedup vs baseline: 3.2462x; 1.0149x over previous
"""ConvCNP encoder kernel for 8 Trainium2 NeuronCores.

Computes, for full inputs X(4,1024,2), Y(4,1024,2), grid(16384,2):
    Gram = exp(-0.5*||grid-X||^2)          (B, G, n)
    FM   = Gram @ [1, Y]                   (B, G, 3)
    out  = [FM0, FM1/FM0, FM2/FM0] -> (B, 3, 128, 128)  (y, x image axes)

Sharding: grid axis G split 8 ways (2048 rows / core = 16 output
x-columns); every core handles all 4 batches; no cross-device reduction.

Per-core device pipeline:
  mm1 (PE):  -0.5*d2 as K=10 bf16 matmuls using a hi/lo split of
             g.x - 0.5|g|^2 - 0.5|x|^2 -> PSUM [n-tile 128, g 512].
             K=10 << 128, so 4 matmuls run concurrently in 32-row
             strips of the PE array via tile_position (inputs are
             replicated at partition offsets 0/32/64/96).
  exp (ACT): PSUM -> SBUF Gram (bf16), batched [128, <=4*512] exps
  mm2 (PE):  Gram[n,g128] stationary x E[n, (1,Yhi,Ylo)] moving,
             accumulated over 8 n-tiles -> FM [g(=y) 128, 5] PSUM
  norm (DVE): density reciprocal + multiplies, fp32
  DMA out:   [y, x] tiles per (b, c)
"""

import numpy as np

B = 4
N = 1024
G = 16384
NCORES = 8
GS = G // NCORES          # 2048 grid rows per core
NT = N // 128             # 8 context tiles
JS = GS // 512            # 4 g-blocks of 512 per core
K = 10                    # contraction rows of the d2 factorization
XCOLS = GS // 128         # 16 output x-columns per core

# combined input layout (free-dim offsets, bf16 elements)
A_W = B * 2 * 128         # 1024
B_W = GS                  # 2048
E_W = B * NT * 5          # 160
IN_W = A_W + B_W + E_W    # 3232

_CACHE = {}


def _build_nc_general():
    import concourse.bacc as bacc
    import concourse.mybir as mybir
    import concourse.tile as tile
    from contextlib import ExitStack

    f32 = mybir.dt.float32
    bf16 = mybir.dt.bfloat16

    nc = bacc.Bacc("TRN2", target_bir_lowering=False, debug=False,
                   num_devices=NCORES)
    in_d = nc.dram_tensor("IN", [128, IN_W], bf16, kind="ExternalInput")
    out_d = nc.dram_tensor("OUT", [B, 3, 128, XCOLS], f32, kind="ExternalOutput")

    EXP = mybir.ActivationFunctionType.Exp

    with tile.TileContext(nc) as tc, ExitStack() as ctx:
        consts = ctx.enter_context(tc.tile_pool(name="consts", bufs=1))
        gram_pool = ctx.enter_context(tc.tile_pool(name="gram", bufs=4))
        mm1a_pool = ctx.enter_context(tc.tile_pool(name="mm1a", bufs=1, space="PSUM"))
        mm1b_pool = ctx.enter_context(tc.tile_pool(name="mm1b", bufs=1, space="PSUM"))
        mm2_pool = ctx.enter_context(tc.tile_pool(name="mm2", bufs=1, space="PSUM"))
        small = ctx.enter_context(tc.tile_pool(name="small", bufs=4))
        outp = ctx.enter_context(tc.tile_pool(name="outp", bufs=1))

        # separate tiles so consumers only wait for the DMA they need;
        # the first mm1 group touches only A[b0] and B[j0] — land those
        # first, one on each queue, before the bulk.
        a0_sb = consts.tile([128, 2 * 128], bf16)
        a123_sb = consts.tile([128, 3 * 2 * 128], bf16)
        b_t = [consts.tile([128, 512], bf16, name=f"bj{j}", tag=f"bj{j}")
               for j in range(JS)]
        e_sb = consts.tile([128, E_W], bf16)

        def in_col(c0, w):
            return in_d[:, c0:c0 + w]

        # IN column layout: [a0 256 | bj0 512 | bj1 512 | a123 768 |
        #                    bj2 512 | bj3 512 | e 160]
        nc.sync.dma_start(out=a0_sb, in_=in_col(0, 256))
        nc.sync.dma_start(out=b_t[0], in_=in_col(256, 512))
        nc.gpsimd.dma_start(out=b_t[1], in_=in_col(768, 512))
        nc.sync.dma_start(out=a123_sb, in_=in_col(1280, 768))
        nc.gpsimd.dma_start(out=b_t[2], in_=in_col(2048, 512))
        nc.sync.dma_start(out=b_t[3], in_=in_col(2560, 512))
        nc.gpsimd.dma_start(out=e_sb, in_=in_col(3072, E_W))

        # views: A[b] -> [128, 2, 128], E [128, B, NT, 5]
        a0_v = a0_sb.rearrange("p (h m) -> p h m", h=2)
        a123_v = a123_sb.rearrange("p (b h m) -> p b h m", b=3, h=2)
        e_v = e_sb.rearrange("p (b t c) -> p b t c", b=B, t=NT)

        def a_slice(b, row, h4):
            if b == 0:
                return a0_v[32 * row:32 * row + K, h4, :]
            return a123_v[32 * row:32 * row + K, b - 1, h4, :]

        def b_slice(j):
            return b_t[j]

        out_sb = outp.tile([128, B, 3, XCOLS], f32)
        grams = {}

        def emit_mm1_group(b, slots, ps):
            """slots: global slot ids (b-local, 0..31), j = s//8, nt = s%8"""
            for i, s in enumerate(slots):
                j = s // 8
                nt = s % 8
                row = nt % 4
                lhsT = a_slice(b, row, nt // 4)
                rhs = b_slice(j)[32 * row:32 * row + K, :]
                nc.tensor.matmul(ps[:, i, :], lhsT, rhs,
                                 start=True, stop=True,
                                 tile_position=(32 * row, 0))

        def emit_mm1_exp(b, h, sizes, tags):
            """one gram tile covering the half's 16 slots (j-outer)"""
            gram = gram_pool.tile([128, 16, 512], bf16, tag="gram",
                                  name=f"gram{b}{h}")
            grams[(b, h)] = gram
            s0 = 0
            for gsz, sel in zip(sizes, tags):
                pool = (mm1a_pool, mm1b_pool)[sel]
                cap = (4, 3)[sel]
                ps = pool.tile([128, cap, 512], f32, tag=f"t{sel}",
                               name=f"ps{sel}")
                emit_mm1_group(b, [16 * h + s0 + i for i in range(gsz)], ps)
                nc.scalar.activation(out=gram[:, s0:s0 + gsz, :],
                                     in_=ps[:, 0:gsz, :], func=EXP)
                s0 += gsz

        def emit_mm1_exp_perj(b, j, sizes, tags):
            """per-j gram tile (8 slots) — used for the final half so the
            tail mm2 only waits on its own j block"""
            gram = gram_pool.tile([128, 8, 512], bf16, tag="gramj",
                                  name=f"gramj{b}{j}")
            grams[("j", b, j)] = gram
            s0 = 0
            for gsz, sel in zip(sizes, tags):
                pool = (mm1a_pool, mm1b_pool)[sel]
                cap = (4, 3)[sel]
                ps = pool.tile([128, cap, 512], f32, tag=f"t{sel}",
                               name=f"ps{sel}")
                emit_mm1_group(b, [8 * j + s0 + i for i in range(gsz)], ps)
                nc.scalar.activation(out=gram[:, s0:s0 + gsz, :],
                                     in_=ps[:, 0:gsz, :], func=EXP)
                s0 += gsz

        def emit_mm2_j(b, j, gram, base):
            fm = grams[("fm", b)]
            for r in range(4):
                gsub = j * 4 + r
                for nt in range(NT):
                    nc.tensor.matmul(
                        fm[:, gsub, :],
                        gram[:, base + nt, r * 128:(r + 1) * 128],
                        e_v[:, b, nt, :],
                        start=(nt == 0),
                        stop=(nt == NT - 1),
                    )

        def emit_norm(b, sl, dma_engine):
            """normalize fm[:, sl, :] and emit ONE combined output DMA
            covering all 3 channels of this gsub range"""
            fm = grams[("fm", b)]
            w = sl.stop - sl.start
            fmc = small.tile([128, 8, 5], f32, tag="fmc")
            nc.vector.tensor_copy(fmc[:, 0:w, :], fm[:, sl, :])
            recip = small.tile([128, 8], f32, tag="recip")
            nc.vector.reciprocal(recip[:, 0:w], fmc[:, 0:w, 0])
            nc.vector.tensor_copy(out_sb[:, b, 0, sl], fmc[:, 0:w, 0])
            v1 = small.tile([128, 8], f32, tag="v1")
            nc.vector.tensor_add(v1[:, 0:w], fmc[:, 0:w, 1], fmc[:, 0:w, 3])
            nc.vector.tensor_mul(out_sb[:, b, 1, sl], v1[:, 0:w], recip[:, 0:w])
            v2 = small.tile([128, 8], f32, tag="v2")
            nc.vector.tensor_add(v2[:, 0:w], fmc[:, 0:w, 2], fmc[:, 0:w, 4])
            nc.vector.tensor_mul(out_sb[:, b, 2, sl], v2[:, 0:w], recip[:, 0:w])
            # dst iterated (y, c, x) to match the SBUF (partition, c, x) order
            dst = out_d[b, :, :, sl].rearrange("c y x -> y c x")
            dma_engine.dma_start(out=dst, in_=out_sb[:, b, :, sl])

        # pattern schedule: first half starts with a 2-slot group so the
        # first exp fires as early as possible; thereafter tags alternate
        # across the boundary so ACT never waits for a psum refill.
        half_patterns = [((2, 4, 3, 4, 3), (1, 0, 1, 0, 1))]
        for k in range(1, 7):
            if k % 2 == 1:
                half_patterns.append(((4, 3, 4, 3, 2), (0, 1, 0, 1, 0)))
            else:
                half_patterns.append(((3, 4, 3, 4, 2), (1, 0, 1, 0, 1)))

        # software pipeline: mm1/exp of (b) overlaps mm2/norm of (b-1)
        for b in range(B):
            fm_t = mm2_pool.tile([128, XCOLS, 5], f32, tag="fm")
            grams[("fm", b)] = fm_t
            if b < B - 1:
                emit_mm1_exp(b, 0, *half_patterns[2 * b])
                emit_mm1_exp(b, 1, *half_patterns[2 * b + 1])
            else:
                emit_mm1_exp(b, 0, *half_patterns[6])
                emit_mm1_exp(b, 1, ((4, 3, 4, 3, 2)), ((0, 1, 0, 1, 0)))
            if b >= 1:
                p = b - 1
                for h in range(2):
                    g = grams[(p, h)]
                    emit_mm2_j(p, 2 * h, g, 0)
                    emit_mm2_j(p, 2 * h + 1, g, 8)
                    emit_norm(p, slice(8 * h, 8 * h + 8), nc.sync)
        b = B - 1
        for h in range(2):
            g = grams[(b, h)]
            emit_mm2_j(b, 2 * h, g, 0)
            emit_mm2_j(b, 2 * h + 1, g, 8)
            emit_norm(b, slice(8 * h, 8 * h + 8), nc.sync)

    nc.compile()
    return nc


# ---------------------------------------------------------------------------
# Separable path: the reference grid is a meshgrid, so the 2-D RBF factors
# into 1-D Gaussians:  Gram[(x,y), n] = A1[x, n] * A2[y, n].
#   FM[y, (x,c)] = sum_n A2[n, y] * (A1[x, n] * E[n, c])
# which needs exp on only ~0.6M elements/core instead of 8.4M.
# ---------------------------------------------------------------------------

KS = 7                     # 1-D factorization rows
S_W = B * NT * 128         # stationary block width (per X component)
INS_W = 2 * S_W + 128 + XCOLS   # G2y | G1x | S2 | S1


def _build_nc_sep():
    import concourse.bacc as bacc
    import concourse.mybir as mybir
    import concourse.tile as tile
    from contextlib import ExitStack

    f32 = mybir.dt.float32
    f16 = mybir.dt.float16

    nc = bacc.Bacc("TRN2", target_bir_lowering=False, debug=False,
                   num_devices=NCORES)
    ins_d = nc.dram_tensor("INS", [KS, INS_W], f16, kind="ExternalInput")
    ine_d = nc.dram_tensor("INE", [128, B * NT * 3], f32, kind="ExternalInput")
    out_d = nc.dram_tensor("OUT", [B, 3, 128, XCOLS], f32, kind="ExternalOutput")

    EXP = mybir.ActivationFunctionType.Exp

    with tile.TileContext(nc) as tc, ExitStack() as ctx:
        consts = ctx.enter_context(tc.tile_pool(name="consts", bufs=1))
        a2_pool = ctx.enter_context(tc.tile_pool(name="a2", bufs=2))
        a1_pool = ctx.enter_context(tc.tile_pool(name="a1", bufs=2))
        w_pool = ctx.enter_context(tc.tile_pool(name="w", bufs=2))
        ps2_pool = ctx.enter_context(tc.tile_pool(name="ps2", bufs=2, space="PSUM"))
        ps1_pool = ctx.enter_context(tc.tile_pool(name="ps1", bufs=2, space="PSUM"))
        fm_pool = ctx.enter_context(tc.tile_pool(name="fmp", bufs=2, space="PSUM"))
        small = ctx.enter_context(tc.tile_pool(name="small", bufs=4))
        outp = ctx.enter_context(tc.tile_pool(name="outp", bufs=1))

        # column layout: [G2 128 | S2b0..S2b3 (1024 each) | G1 16 |
        #                 S1b0..S1b3 (1024 each)]
        # per-b tiles so batch 0's matmuls start as soon as its slice lands
        gs0 = consts.tile([KS, 128 + 1024], f16, name="gs0")   # G2 + S2b0
        s2b = [consts.tile([KS, 1024], f16, name=f"s2b{b}") for b in (1, 2, 3)]
        gs1 = consts.tile([KS, 16 + 1024], f16, name="gs1")    # G1 + S1b0
        s1b = [consts.tile([KS, 1024], f16, name=f"s1b{b}") for b in (1, 2, 3)]
        e_sb = consts.tile([128, B * NT * 3], f32)
        nc.sync.dma_start(out=gs0, in_=ins_d[:, 0:1152])
        nc.gpsimd.dma_start(out=gs1, in_=ins_d[:, 4224:5264])
        nc.sync.dma_start(out=s2b[0], in_=ins_d[:, 1152:2176])
        nc.gpsimd.dma_start(out=s1b[0], in_=ins_d[:, 5264:6288])
        nc.sync.dma_start(out=s2b[1], in_=ins_d[:, 2176:3200])
        nc.sync.dma_start(out=s2b[2], in_=ins_d[:, 3200:4224])
        nc.gpsimd.dma_start(out=s1b[1], in_=ins_d[:, 6288:7312])
        nc.gpsimd.dma_start(out=s1b[2], in_=ins_d[:, 7312:8336])
        nc.gpsimd.dma_start(out=e_sb, in_=ine_d[:])

        g2_v = gs0[:, 0:128]
        g1_v = gs1[:, 0:16]

        def s2_slice(b, nt):
            t = gs0[:, 128:] if b == 0 else s2b[b - 1]
            return t[:, nt * 128:(nt + 1) * 128]

        def s1_slice(b, nt):
            t = gs1[:, 16:] if b == 0 else s1b[b - 1]
            return t[:, nt * 128:(nt + 1) * 128]
        e_v = e_sb.rearrange("p (b t c) -> p b t c", b=B, t=NT)

        out_sb = outp.tile([128, B, 3, XCOLS], f32)
        st = {}

        def emit_b(b):
            # A2 = exp(-0.5 (gy - X1)^2): [n(128), 8nt, y(128)] fp16
            a2 = a2_pool.tile([128, NT, 128], f16, tag="a2", name=f"a2_{b}")
            ps2 = ps2_pool.tile([128, NT, 128], f32, tag="ps2", name="ps2")
            for nt in range(NT):
                nc.tensor.matmul(ps2[:, nt, :], s2_slice(b, nt), g2_v,
                                 start=True, stop=True)
            nc.scalar.activation(out=a2, in_=ps2, func=EXP)
            # A1 = exp(-0.5 (gx - X0)^2): [n(128), 8nt, x(16)] fp16
            a1 = a1_pool.tile([128, NT, XCOLS], f16, tag="a1", name=f"a1_{b}")
            ps1 = ps1_pool.tile([128, NT, XCOLS], f32, tag="ps1", name="ps1")
            for nt in range(NT):
                nc.tensor.matmul(ps1[:, nt, :], s1_slice(b, nt), g1_v,
                                 start=True, stop=True)
            nc.scalar.activation(out=a1, in_=ps1, func=EXP)
            # W[n, nt, x, c] = A1[n, nt, x] * E[n, nt, c] — one broadcast
            # multiply over the whole batch
            w = w_pool.tile([128, NT, XCOLS, 3], f16, tag="w", name=f"w_{b}")
            a1_b = a1[:, :, :, None].to_broadcast([128, NT, XCOLS, 3])
            e_b = e_v[:, b, :, None, :].to_broadcast([128, NT, XCOLS, 3])
            nc.vector.tensor_mul(w, a1_b, e_b)
            # FM[y, x, c] = sum_nt A2[:, nt, :].T @ W[:, nt, :, :]
            fm = fm_pool.tile([128, XCOLS, 3], f32, tag="fm", name=f"fm_{b}")
            st[b] = fm
            for nt in range(NT):
                nc.tensor.matmul(fm[:], a2[:, nt, :], w[:, nt, :, :],
                                 start=(nt == 0), stop=(nt == NT - 1))

        def emit_norm(b):
            fm = st[b]
            fmc = small.tile([128, XCOLS, 3], f32, tag="fmc")
            nc.vector.tensor_copy(fmc, fm)
            recip = small.tile([128, XCOLS], f32, tag="recip")
            nc.vector.reciprocal(recip, fmc[:, :, 0])
            nc.vector.tensor_copy(out_sb[:, b, 0, :], fmc[:, :, 0])
            nc.vector.tensor_mul(out_sb[:, b, 1, :], fmc[:, :, 1], recip)
            nc.vector.tensor_mul(out_sb[:, b, 2, :], fmc[:, :, 2], recip)
            dst = out_d[b].rearrange("c y x -> y c x")
            nc.sync.dma_start(out=dst, in_=out_sb[:, b, :, :])

        for b in range(B):
            emit_b(b)
            if b >= 1:
                emit_norm(b - 1)
        emit_norm(B - 1)

    nc.compile()
    return nc


def _sep_factors(gv, xc):
    """K=7 fp16 factorization of -0.5 (g - x)^2 along one dimension.
    Returns A (B, 7, N) stationary rows and Bm (7, len(gv)) moving rows."""
    f16 = np.float16

    def split(a):
        hi = a.astype(f16).astype(np.float32)
        lo = (a - hi).astype(f16).astype(np.float32)
        return hi, lo

    sx = -0.5 * xc * xc
    sg = -0.5 * gv * gv
    xh, xl = split(xc)
    gh, gl = split(gv)
    sxh, sxl = split(sx)
    sgh, sgl = split(sg)
    on = np.ones_like(xc)
    og = np.ones_like(gv)
    A = np.stack([xh, xl, xh, sxh, sxl, on, on], axis=1)
    Bm = np.stack([gh, gh, gl, og, og, sgh, sgl], axis=0)
    return A.astype(f16), Bm.astype(f16)


def _prepare_inputs_sep(X, Y, grid):
    f16 = np.float16
    X = np.asarray(X, np.float32)
    Y = np.asarray(Y, np.float32)
    grid = np.asarray(grid, np.float32)
    gxv = grid[::128, 0]
    gyv = grid[:128, 1]

    A2s, G2 = _sep_factors(gyv, X[..., 1])   # (B,7,N), (7,128)
    A1s, G1 = _sep_factors(gxv, X[..., 0])   # (B,7,N), (7,128x_all)
    # stationary blocks [7, B, NT, 128]
    s2 = A2s.transpose(1, 0, 2).reshape(KS, B, NT, 128)
    s1 = A1s.transpose(1, 0, 2).reshape(KS, B, NT, 128)
    E = np.concatenate([np.ones((B, N, 1), np.float32), Y], axis=-1)
    ine = np.ascontiguousarray(
        E.reshape(B, NT, 128, 3).transpose(2, 0, 1, 3).reshape(128, B * NT * 3))

    in_maps = []
    s2f = s2.reshape(KS, S_W)
    s1f = s1.reshape(KS, S_W)
    for c in range(NCORES):
        g1c = G1[:, c * XCOLS:(c + 1) * XCOLS]
        ins = np.concatenate([G2, s2f, g1c, s1f], axis=1).astype(f16)
        in_maps.append({"INS": np.ascontiguousarray(ins), "INE": ine})
    return in_maps


def _grid_separable(grid):
    grid = np.asarray(grid)
    if grid.shape != (G, 2):
        return False
    gxv = grid[::128, 0]
    gyv = grid[:128, 1]
    return (np.array_equal(grid[:, 0], np.repeat(gxv, 128))
            and np.array_equal(grid[:, 1], np.tile(gyv, 128)))


def _split_hi_lo(a):
    import ml_dtypes

    bf = ml_dtypes.bfloat16
    hi = a.astype(bf).astype(np.float32)
    lo = (a - hi).astype(bf).astype(np.float32)
    return hi, lo


def _prepare_inputs(X, Y, grid):
    """Host-side packing: per-core input maps for the SPMD kernel."""
    import ml_dtypes

    bf = ml_dtypes.bfloat16
    X = np.asarray(X, np.float32)
    Y = np.asarray(Y, np.float32)
    grid = np.asarray(grid, np.float32)

    sx = -0.5 * np.sum(X * X, axis=-1)        # (B, N)
    sg = -0.5 * np.sum(grid * grid, axis=-1)  # (G,)
    xh, xl = _split_hi_lo(X)
    gh, gl = _split_hi_lo(grid)
    sxh, sxl = _split_hi_lo(sx)
    sgh, sgl = _split_hi_lo(sg)
    ones_n = np.ones((B, N), np.float32)
    ones_g = np.ones((G,), np.float32)

    # M'[n, g] = sum_k A[k, n] * Bm[k, g] = g.x - 0.5|x|^2 - 0.5|g|^2
    A = np.stack(
        [xh[..., 0], xh[..., 1], xl[..., 0], xl[..., 1],
         xh[..., 0], xh[..., 1], sxh, sxl, ones_n, ones_n],
        axis=1,
    )  # (B, K, N)
    Bm = np.stack(
        [gh[:, 0], gh[:, 1], gh[:, 0], gh[:, 1],
         gl[:, 0], gl[:, 1], ones_g, ones_g, sgh, sgl],
        axis=0,
    )  # (K, G)

    # A replicated into 4 row-strips: strip i (partitions 32i..32i+9)
    # holds A rows for nt = h*4 + i  -> [128, B, 2, 128]
    A4 = A.transpose(1, 0, 2).reshape(K, B, 2, 4, 128)  # k b h i p
    arep = np.zeros((128, B, 2, 128), np.float32)
    for i in range(4):
        arep[32 * i:32 * i + K] = A4[:, :, :, i, :]

    # E: [128, B, NT, 5] = [1, yh0, yh1, yl0, yl1]
    yh, yl = _split_hi_lo(Y)
    E = np.stack([ones_n, yh[..., 0], yh[..., 1], yl[..., 0], yl[..., 1]],
                 axis=-1)
    ey = E.reshape(B, NT, 128, 5).transpose(2, 0, 1, 3)

    in_maps = []
    ar = arep.reshape(128, A_W)
    for c in range(NCORES):
        # B replicated into the same 4 row-strips
        brep = np.zeros((128, GS), np.float32)
        for i in range(4):
            brep[32 * i:32 * i + K] = Bm[:, c * GS:(c + 1) * GS]
        # column order: [a0 | bj0 | bj1 | a123 | bj2 | bj3 | e]
        packed = np.concatenate(
            [ar[:, 0:256], brep[:, 0:512], brep[:, 512:1024],
             ar[:, 256:A_W], brep[:, 1024:1536], brep[:, 1536:2048],
             ey.reshape(128, E_W)], axis=1)
        in_maps.append({"IN": np.ascontiguousarray(packed).astype(bf)})
    return in_maps


def _run(in_maps, builder, key, trace=False):
    from concourse.bass_utils import run_bass_kernel_spmd

    if key not in _CACHE:
        _CACHE[key] = builder()
    nc = _CACHE[key]
    return run_bass_kernel_spmd(nc, in_maps, core_ids=list(range(NCORES)),
                                trace=trace)


def kernel(X, Y, grid, _trace=False, _results_out=None):
    if _grid_separable(grid):
        in_maps = _prepare_inputs_sep(X, Y, grid)
        res = _run(in_maps, _build_nc_sep, "sep", trace=_trace)
    else:
        in_maps = _prepare_inputs(X, Y, grid)
        res = _run(in_maps, _build_nc_general, "gen", trace=_trace)
    out = np.empty((B, 3, 128, 128), np.float32)
    for c in range(NCORES):
        out[:, :, :, c * XCOLS:(c + 1) * XCOLS] = res.results[c]["OUT"]
    if _results_out is not None:
        _results_out.append(res)
    return out
